# revision 38
# baseline (speedup 1.0000x reference)
"""Distributed causal RoPE attention for Trainium2 (8 NeuronCores).

Problem: nn_CausalRpeAttn — B=2, S=2048, D=1024, H=16, Dh=64, fp32.

Sharding (data + head parallel): core c handles batch c//4 and heads
4*(c%4) .. 4*(c%4)+3 (a 256-wide feature slice). Wq/Wk/Wv are split
column-wise (by output head group), Wo row-wise. Each core writes its
full [1024, 2048] (transposed) bf16 partial output projection (with
bo/4 pre-added); the host unshards by summing the 4 partials per batch
and transposing back. Attention itself is fully independent per
(batch, head), so the only cross-core combination is that final sum.

Performance structure (v3):
 - Everything bf16 on the wires; fp32 only in PSUM accumulation and the
   softmax denominator reciprocal.
 - The Scalar engine runs (almost) nothing but the softmax exp; it is
   the phase-B bottleneck, so attention q-tiles are INTERLEAVED into
   the projection phase: q0,k0 proj -> v(first quarter) -> qt0-pair0
   scores (exp stream starts ~30us in) -> more v -> q1,k1 -> etc.
 - q/k projections feature-major (moving 512); v projected
   POSITION-major on the PE (x chunk stationary, Wv moving) - no
   transposes; bv added during the DVE PSUM->SBUF evacuation against a
   pre-broadcast bias tile.
 - Scores transposed sT[k, q]; the two heads of a 128-feature block run
   concurrently on disjoint PE row groups into different PSUM banks.
   Causal-trimmed moving ranges everywhere.
 - Diagonal-block causal masking via DVE tensor_mask against
   host-provided column-index/partition-threshold constants (GpSimd
   would pay a ~5us ucode reconfig each time it alternates
   affine_select with partition_broadcast).
 - v carries an appended ones-row so PV emits the softmax denominator
   row; denominators are copied to SBUF, inverted with the single-slot
   reciprocal_approx_fast (~5x faster than the iterative divide),
   partition-broadcast on GpSimd (its only compute), and two DVE muls
   produce the bf16 Wo input. Wo runs one q-tile late to hide that
   chain; the final q-tile's output casts alternate DVE/ACT to shorten
   the tail.
 - Input DMAs are split small and issued from sync/scalar/gpsimd
   queues so the first matmul starts ~6us in; rope swap DMAs ride the
   sync queue to keep the Scalar queue free for exp.
"""

import os
import ml_dtypes
import numpy as np

B, S, D, H, DH = 2, 2048, 1024, 16, 64
N_CORES = 8
FPC = 256  # features per core (4 heads)
QT = 512
NQT = S // QT  # 4
NST = S // 512  # 4 s-tiles for projections

_cache = {}
last_run_info = {}


def _build():
    import concourse.bass as bass
    import concourse.mybir as mybir
    import concourse.tile as tile
    from concourse import bacc

    F32 = mybir.dt.float32
    BF16 = mybir.dt.bfloat16
    EXP = mybir.ActivationFunctionType.Exp
    IDENT = mybir.ActivationFunctionType.Identity

    nc = bacc.Bacc("TRN2", target_bir_lowering=False, debug=False,
                   num_devices=N_CORES)

    qkvT_e = nc.dram_tensor("qkvT", [D, S], BF16, kind="ExternalInput").ap()
    wq_e = nc.dram_tensor("wq", [D, FPC], BF16, kind="ExternalInput").ap()
    wk_e = nc.dram_tensor("wk", [D, FPC], BF16, kind="ExternalInput").ap()
    wv_e = nc.dram_tensor("wv", [D, FPC], BF16, kind="ExternalInput").ap()
    wo_e = nc.dram_tensor("wo", [FPC, D], BF16, kind="ExternalInput").ap()
    bq_e = nc.dram_tensor("bq", [FPC], F32, kind="ExternalInput").ap()
    bk_e = nc.dram_tensor("bk", [FPC], F32, kind="ExternalInput").ap()
    bv_e = nc.dram_tensor("bv", [1, FPC], F32, kind="ExternalInput").ap()
    bo_e = nc.dram_tensor("bo", [D], F32, kind="ExternalInput").ap()
    cos2_e = nc.dram_tensor("cos2", [128, S], BF16, kind="ExternalInput").ap()
    sinx_e = nc.dram_tensor("sinx", [128, S], BF16, kind="ExternalInput").ap()
    mofs_e = nc.dram_tensor("mofs", [128, FPC], F32, kind="ExternalInput").ap()
    mlen_e = nc.dram_tensor("mlen", [128, 1], F32, kind="ExternalInput").ap()
    out_e = nc.dram_tensor("out", [D, S], BF16, kind="ExternalOutput").ap()

    from contextlib import ExitStack
    with tile.TileContext(nc) as tc:
        with ExitStack() as ctx:
            ep = ctx.enter_context
            consts = ep(tc.tile_pool(name="consts", bufs=1))
            xin_pool = ep(tc.tile_pool(name="xin", bufs=1))
            rope_pool = ep(tc.tile_pool(name="rope", bufs=4))
            qb_pool = ep(tc.tile_pool(name="qb", bufs=2))
            qbs_pool = ep(tc.tile_pool(name="qbs", bufs=2))
            tmp_pool = ep(tc.tile_pool(name="tmp", bufs=2))
            vsb_pool = ep(tc.tile_pool(name="vsb", bufs=1))
            probs_pool = ep(tc.tile_pool(name="probs", bufs=4))
            woin_pool = ep(tc.tile_pool(name="woin", bufs=2))
            rec_pool = ep(tc.tile_pool(name="rec", bufs=2))
            rb_pool = ep(tc.tile_pool(name="rb", bufs=2))
            osb_pool = ep(tc.tile_pool(name="osb", bufs=3))
            # PSUM: sc 2x[128,1024] = 4 banks, ps 4x[128,512] = 4 banks.
            sc_pool = ep(tc.tile_pool(name="sc", bufs=2, space="PSUM"))
            ps_pool = ep(tc.tile_pool(name="ps", bufs=4, space="PSUM"))

            # ---- input DMAs, split fine and spread across issue queues ----
            wq_sb = consts.tile([128, 8, FPC], BF16, tag="wq")
            wk_sb = consts.tile([128, 8, FPC], BF16, tag="wk")
            wv_sb = consts.tile([128, 8, FPC], BF16, tag="wv")
            wq_r = wq_e.rearrange("(kt p) f -> p kt f", p=128)
            wk_r = wk_e.rearrange("(kt p) f -> p kt f", p=128)
            wv_r = wv_e.rearrange("(kt p) f -> p kt f", p=128)
            x_all = xin_pool.tile([128, 8, S], BF16, tag="x")

            # sync queue: wq interleaved with the first x quarter so the
            # q-projection starts streaming immediately; swap DMAs and
            # output DMAs ride this queue later (program order).
            for kt in range(8):
                nc.sync.dma_start(out=wq_sb[:, kt, :], in_=wq_r[:, kt, :])
                nc.sync.dma_start(out=x_all[:, kt, 0:512],
                                  in_=qkvT_e[kt * 128:(kt + 1) * 128, 0:512])
            for kt in range(8):
                nc.sync.dma_start(out=x_all[:, kt, 512:1024],
                                  in_=qkvT_e[kt * 128:(kt + 1) * 128,
                                             512:1024])

            # scalar queue: ONLY small/early DMAs — anything more clogs
            # the in-order Scalar queue and delays the exp stream (each
            # DMA issue occupies the queue ~0.6us and waits for slots).
            b_sbs = []
            for name, be in (("bq", bq_e), ("bk", bk_e)):
                t = consts.tile([128, 2], F32, tag=name, name=name)
                nc.scalar.dma_start(out=t[:],
                                    in_=be.rearrange("(t p) -> p t", p=128))
                b_sbs.append(t)
            bq_sb, bk_sb = b_sbs
            # only the h0 halves of the rope tables ride the scalar
            # queue (needed ~19us); h1 goes via gpsimd so the early rope
            # swap DMAs get scalar DMA-queue slots sooner.
            cos2_sb = consts.tile([128, S], BF16, tag="cos2")
            sinx_sb = consts.tile([128, S], BF16, tag="sinx")
            nc.scalar.dma_start(out=cos2_sb[:, 0:1024], in_=cos2_e[:, 0:1024])
            nc.scalar.dma_start(out=sinx_sb[:, 0:1024], in_=sinx_e[:, 0:1024])

            # gpsimd queue: bv + mask consts first, wv/wk (needed ~14us),
            # then x quarters 2,3 and the rest.
            bv_row = consts.tile([1, FPC], F32, tag="bvr")
            nc.gpsimd.dma_start(out=bv_row[:], in_=bv_e)
            nc.gpsimd.dma_start(out=cos2_sb[:, 1024:2048],
                                in_=cos2_e[:, 1024:2048])
            nc.gpsimd.dma_start(out=sinx_sb[:, 1024:2048],
                                in_=sinx_e[:, 1024:2048])
            mofs_sb = consts.tile([128, FPC], F32, tag="mofs")
            nc.gpsimd.dma_start(out=mofs_sb[:], in_=mofs_e)
            mlen_sb = consts.tile([128, 1], F32, tag="mlen")
            nc.gpsimd.dma_start(out=mlen_sb[:], in_=mlen_e)
            for kt in range(0, 8, 4):
                nc.gpsimd.dma_start(out=wv_sb[:, kt:kt + 4, :],
                                    in_=wv_r[:, kt:kt + 4, :])
            for kt in range(0, 8, 4):
                nc.gpsimd.dma_start(out=wk_sb[:, kt:kt + 4, :],
                                    in_=wk_r[:, kt:kt + 4, :])
            for kt in range(8):
                nc.gpsimd.dma_start(
                    out=x_all[:, kt, 1024:2048],
                    in_=qkvT_e[kt * 128:(kt + 1) * 128, 1024:2048])
            wo_sb = consts.tile([128, 2, D], BF16, tag="wo")
            wo_r = wo_e.rearrange("(pt p) f -> p pt f", p=128)
            for pt in range(2):
                nc.gpsimd.dma_start(out=wo_sb[:, pt, :], in_=wo_r[:, pt, :])
            bo_sb = consts.tile([128, 8], F32, tag="bo")
            nc.gpsimd.dma_start(out=bo_sb[:],
                                in_=bo_e.rearrange("(t p) -> p t", p=128))
            bv_bc = consts.tile([128, FPC], F32, tag="bvb")
            nc.gpsimd.partition_broadcast(bv_bc[:], bv_row[0:1, :])


            # v with ones row: [pos 128, 16 s-tiles, 4 heads, 64+1]
            v_sb = vsb_pool.tile([128, 16, 4, DH + 1], BF16, tag="v")
            nc.vector.memset(v_sb[:].rearrange("p a b c -> p (a b c)"), 1.0)

            # rope targets: [feat 128, S] per partition-tile, bf16
            qrot = [rope_pool.tile([128, S], BF16, tag="rope", name=f"qrot{i}")
                    for i in range(2)]
            krot = [rope_pool.tile([128, S], BF16, tag="rope", name=f"krot{i}")
                    for i in range(2)]

            def proj_st(qb, w_sb, b_sb, pt, st, on_act):
                ss = slice(st * 512, (st + 1) * 512)
                ps = ps_pool.tile([128, 512], F32, tag="ps", name="psp")
                for kt in range(8):
                    nc.tensor.matmul(
                        ps[:], w_sb[:, kt, pt * 128:(pt + 1) * 128],
                        x_all[:, kt, ss],
                        start=(kt == 0), stop=(kt == 7))
                if on_act:
                    nc.scalar.activation(out=qb[:, ss], in_=ps[:],
                                         func=IDENT,
                                         bias=b_sb[:, pt:pt + 1])
                else:
                    nc.vector.tensor_scalar_add(
                        out=qb[:, ss], in0=ps[:],
                        scalar1=b_sb[:, pt:pt + 1])

            def proj_block(w_sb, b_sb, pt, on_act):
                # q/k projection for one 128-feature block -> qb bf16
                qb = qb_pool.tile([128, S], BF16, tag="qb", name="qb")
                for st in range(NST):
                    proj_st(qb, w_sb, b_sb, pt, st, on_act)
                return qb

            def rope_half(qb, qbs, dst, h, early=False):
                # dst = qb*cos + swap32(qb)*sinx for one free-dim half.
                # The front ropes issue their swaps on the scalar queue
                # (idle before the exp stream starts); later ones ride
                # sync so they never delay an exp.
                eng = nc.scalar if early else nc.sync
                hs = slice(h * 1024, (h + 1) * 1024)
                nc.vector.tensor_mul(out=dst[:, hs], in0=qb[:, hs],
                                     in1=cos2_sb[:, hs])
                for blk in (0, 1):
                    p0 = blk * 64
                    eng.dma_start(out=qbs[p0:p0 + 32, hs],
                                  in_=qb[p0 + 32:p0 + 64, hs])
                    eng.dma_start(out=qbs[p0 + 32:p0 + 64, hs],
                                  in_=qb[p0:p0 + 32, hs])
                tmp = tmp_pool.tile([128, 1024], BF16, tag="tmp",
                                    name="tmp")
                nc.vector.tensor_mul(out=tmp[:], in0=qbs[:, hs],
                                     in1=sinx_sb[:, hs])
                nc.vector.tensor_add(out=dst[:, hs], in0=dst[:, hs],
                                     in1=tmp[:])

            def v_block(sti):
                # v projected position-major: x chunk stationary, Wv moving
                pv_ps = ps_pool.tile([128, FPC], F32, tag="ps", name="pv_ps")
                for kt in range(8):
                    nc.tensor.matmul(
                        pv_ps[:], x_all[:, kt, sti * 128:(sti + 1) * 128],
                        wv_sb[:, kt, :],
                        start=(kt == 0), stop=(kt == 7))
                nc.vector.tensor_add(
                    out=v_sb[:, sti, :, 0:DH],
                    in0=pv_ps[:].rearrange("p (h d) -> p h d", h=4),
                    in1=bv_bc[:].rearrange("p (h d) -> p h d", h=4))

            woin = [woin_pool.tile([128, S], BF16, tag="woin",
                                   name=f"woin{i}") for i in range(2)]

            def scores(kt, qt, pair, qt0):
                ksl = slice(kt * 128, (kt + 1) * 128)
                off = max(0, kt * 128 - qt * 512)
                ps_s = sc_pool.tile([128, 1024], F32, tag="sc", name="ps_s")
                psv = ps_s[:].rearrange("p (h q) -> p h q", h=2)
                for h in (0, 1):
                    nc.tensor.matmul(
                        psv[:, h, off:512],
                        krot[pair][h * 64:(h + 1) * 64, ksl],
                        qrot[pair][h * 64:(h + 1) * 64, qt0 + off:qt0 + 512],
                        start=True, stop=True)
                pr = probs_pool.tile([128, 1024], BF16, tag="pr", name="pr")
                prv = pr[:].rearrange("p (h q) -> p h q", h=2)
                nc.scalar.activation(out=prv[:, :, off:512],
                                     in_=psv[:, :, off:512],
                                     func=EXP, scale=0.125)
                if kt * 128 >= qt * 512:
                    # zero probs above the diagonal of this 128-block:
                    # keep column qc (0..127 per head) iff qc >= partition;
                    # TENSOR_MASK: out = in0 if (in1 + imm2) < s0 else 0
                    from concourse.dve_ops import TENSOR_MASK
                    for h in (0, 1):
                        nc.vector._custom_dve(
                            TENSOR_MASK,
                            out=prv[:, h, off:off + 128],
                            in0=prv[:, h, off:off + 128],
                            in1=mofs_sb[:, 0:128],
                            s0=mlen_sb[:, 0:1],
                            imm2=0.0)
                return pr

            def pv(kt, pr, pv_a, pv_b, pair, nkt, qt):
                off = max(0, kt * 128 - qt * 512)
                prv = pr[:].rearrange("p (h q) -> p h q", h=2)
                nc.tensor.matmul(
                    pv_a[0:DH + 1, off:512], v_sb[:, kt, 2 * pair, :],
                    prv[:, 0, off:512],
                    start=(kt == 0), stop=(kt == nkt - 1))
                nc.tensor.matmul(
                    pv_b[0:DH + 1, off:512], v_sb[:, kt, 2 * pair + 1, :],
                    prv[:, 1, off:512],
                    start=(kt == 0), stop=(kt == nkt - 1))

            def attn_pair(qt, pair, fillers=()):
                # scores+pv pipeline for one (q-tile, head-pair). fillers
                # are zero-arg callables emitting independent PE work
                # (v blocks / next projection tiles), woven one per kt so
                # the PE stays busy while the exp stream catches up.
                fillers = list(fillers)
                qt0 = qt * 512
                qsl = slice(qt0, qt0 + 512)
                pv_a = ps_pool.tile([DH + 1, 512], F32, tag="ps",
                                    name="pv_a")
                pv_b = ps_pool.tile([DH + 1, 512], F32, tag="ps",
                                    name="pv_b")
                nkt = 4 * qt + 4
                pr_prev = scores(0, qt, pair, qt0)
                if fillers:
                    fillers.pop(0)()
                for kt in range(1, nkt):
                    pr_k = scores(kt, qt, pair, qt0)
                    if fillers:
                        fillers.pop(0)()
                    pv(kt - 1, pr_prev, pv_a, pv_b, pair, nkt, qt)
                    pr_prev = pr_k
                pv(nkt - 1, pr_prev, pv_a, pv_b, pair, nkt, qt)
                for f in fillers:
                    f()

                # denominator reciprocal + broadcast + normalize
                den = rec_pool.tile([1, 1024], F32, tag="den", name="den")
                nc.vector.tensor_copy(out=den[0:1, 0:512],
                                      in_=pv_a[DH:DH + 1, :])
                nc.vector.tensor_copy(out=den[0:1, 512:1024],
                                      in_=pv_b[DH:DH + 1, :])
                rec = rec_pool.tile([1, 1024], F32, tag="rec", name="rec")
                nc.vector.reciprocal_approx_fast(
                    out=rec[0:1, :], in_=den[0:1, :])
                rb = rb_pool.tile([128, 1024], F32, tag="rb", name="rb")
                nc.gpsimd.partition_broadcast(rb[:], rec[0:1, :])
                nc.vector.tensor_mul(out=woin[pair][0:64, qsl],
                                     in0=pv_a[0:DH, :],
                                     in1=rb[0:64, 0:512])
                nc.vector.tensor_mul(out=woin[pair][64:128, qsl],
                                     in0=pv_b[0:DH, :],
                                     in1=rb[64:128, 512:1024])

            def wo_block(qt, last=False):
                qsl = slice(qt * 512, (qt + 1) * 512)
                for dm in range(8):
                    ps_o = ps_pool.tile([128, 512], F32, tag="ps",
                                        name="ps_o")
                    for pt in range(2):
                        nc.tensor.matmul(
                            ps_o[:], wo_sb[:, pt, dm * 128:(dm + 1) * 128],
                            woin[pt][:, qsl], start=(pt == 0), stop=(pt == 1))
                    ot = osb_pool.tile([128, QT], BF16, tag="ot", name="ot")
                    if last and dm % 2 == 1:
                        nc.scalar.activation(out=ot[:], in_=ps_o[:],
                                             func=IDENT,
                                             bias=bo_sb[:, dm:dm + 1])
                    else:
                        nc.vector.tensor_scalar_add(
                            out=ot[:], in0=ps_o[:],
                            scalar1=bo_sb[:, dm:dm + 1])
                    if last:
                        q0_ = qt * 512
                        nc.sync.dma_start(
                            out=out_e[dm * 128:(dm + 1) * 128,
                                      q0_:q0_ + 256], in_=ot[:, 0:256])
                        nc.sync.dma_start(
                            out=out_e[dm * 128:(dm + 1) * 128,
                                      q0_ + 256:q0_ + 512], in_=ot[:, 256:512])
                    else:
                        nc.sync.dma_start(
                            out=out_e[dm * 128:(dm + 1) * 128, qsl],
                            in_=ot[:])

            # ---- interleaved schedule ----
            # Fine-grained front: the first position-halves of the q0/k0
            # projections and their RoPE go first so qt0-pair0 scores (and
            # the exp stream) start ~20us in. Everything else — remaining
            # projection tiles, RoPE halves, v blocks — is woven into the
            # pair-0 attention as PE filler. Then pair 1 of every q-tile
            # (order 3,0,1,2) with the Wo blocks one step behind.
            qbq0 = qb_pool.tile([128, S], BF16, tag="qb", name="qb_q0")
            qbk0 = qb_pool.tile([128, S], BF16, tag="qb", name="qb_k0")
            qsq0 = qbs_pool.tile([128, S], BF16, tag="qbs", name="qs_q0")
            qsk0 = qbs_pool.tile([128, S], BF16, tag="qbs", name="qs_k0")
            qbq1 = qb_pool.tile([128, S], BF16, tag="qbx", name="qb_q1")
            qbk1 = qb_pool.tile([128, S], BF16, tag="qbx", name="qb_k1")
            qsq1 = qbs_pool.tile([128, S], BF16, tag="qbsx", name="qs_q1")
            qsk1 = qbs_pool.tile([128, S], BF16, tag="qbsx", name="qs_k1")

            proj_st(qbq0, wq_sb, bq_sb, 0, 0, True)
            v_block(0)
            v_block(1)
            proj_st(qbq0, wq_sb, bq_sb, 0, 1, True)
            v_block(2)
            v_block(3)
            rope_half(qbq0, qsq0, qrot[0], 0, early=True)
            proj_st(qbk0, wk_sb, bk_sb, 0, 0, True)
            v_block(4)
            v_block(5)
            proj_st(qbk0, wk_sb, bk_sb, 0, 1, True)
            rope_half(qbk0, qsk0, krot[0], 0, early=True)
            v_block(6)
            v_block(7)

            fill = [
                # qt0 pair0 (4 kts)
                lambda: proj_st(qbq0, wq_sb, bq_sb, 0, 2, True),
                lambda: proj_st(qbk0, wk_sb, bk_sb, 0, 2, True),
                lambda: v_block(8),
                lambda: v_block(9),
                # qt1 pair0 (8 kts)
                lambda: proj_st(qbq0, wq_sb, bq_sb, 0, 3, True),
                lambda: proj_st(qbk0, wk_sb, bk_sb, 0, 3, True),
                lambda: rope_half(qbq0, qsq0, qrot[0], 1),
                lambda: rope_half(qbk0, qsk0, krot[0], 1),
                lambda: v_block(10),
                lambda: v_block(11),
                # qt2 pair0 (12 kts): v12..15 first (qt3's pv needs them)
                lambda: v_block(12),
                lambda: v_block(13),
                lambda: v_block(14),
                lambda: v_block(15),
                lambda: proj_st(qbq1, wq_sb, bq_sb, 1, 0, False),
                lambda: proj_st(qbq1, wq_sb, bq_sb, 1, 1, False),
                lambda: rope_half(qbq1, qsq1, qrot[1], 0),
                lambda: proj_st(qbq1, wq_sb, bq_sb, 1, 2, False),
                lambda: proj_st(qbq1, wq_sb, bq_sb, 1, 3, False),
                lambda: rope_half(qbq1, qsq1, qrot[1], 1),
                # qt3 pair0 (16 kts): k1 h0 only — k1 h1 moves into the
                # pair-1 phase where the PE otherwise idles
                lambda: proj_st(qbk1, wk_sb, bk_sb, 1, 0, False),
                lambda: proj_st(qbk1, wk_sb, bk_sb, 1, 1, False),
                lambda: rope_half(qbk1, qsk1, krot[1], 0),
            ]
            attn_pair(0, 0, fillers=fill[0:4])
            attn_pair(1, 0, fillers=fill[4:10])
            attn_pair(2, 0, fillers=fill[10:20])
            attn_pair(3, 0, fillers=fill[20:])

            fill1 = [
                lambda: proj_st(qbk1, wk_sb, bk_sb, 1, 2, False),
                lambda: proj_st(qbk1, wk_sb, bk_sb, 1, 3, False),
                lambda: rope_half(qbk1, qsk1, krot[1], 1),
            ]

            attn_pair(0, 1, fillers=fill1)
            attn_pair(3, 1)
            wo_block(0)
            attn_pair(2, 1)
            wo_block(3)
            attn_pair(1, 1)
            wo_block(2)
            wo_block(1, last=True)

    nc.compile()
    return nc


def kernel(qkv, cos, sin, Wq, bq, Wk, bk, Wv, bv, Wo, bo):
    from concourse.bass_utils import run_bass_kernel_spmd

    qkv = np.asarray(qkv, dtype=np.float32)
    cos = np.asarray(cos, dtype=np.float32)
    sin = np.asarray(sin, dtype=np.float32)
    Wq, bq = np.asarray(Wq, np.float32), np.asarray(bq, np.float32)
    Wk, bk = np.asarray(Wk, np.float32), np.asarray(bk, np.float32)
    Wv, bv = np.asarray(Wv, np.float32), np.asarray(bv, np.float32)
    Wo, bo = np.asarray(Wo, np.float32), np.asarray(bo, np.float32)

    if "nc" not in _cache:
        _cache["nc"] = _build()
    nc = _cache["nc"]

    bf = ml_dtypes.bfloat16
    cos2 = np.ascontiguousarray(np.tile(cos.T, (2, 1)).astype(bf))  # [128, S]
    sinx = np.tile(sin.T, (2, 1))
    sinx[0:32] *= -1.0
    sinx[64:96] *= -1.0
    sinx = np.ascontiguousarray(sinx.astype(bf))

    # tensor_mask consts: keep col qc (per 128-head-block) iff qc >= p
    #   keep iff mofs[p, qc] + 0 < mlen[p]  with mofs = -qc, mlen = 1-p
    qc = np.arange(128, dtype=np.float32)
    mofs = np.ascontiguousarray(np.tile(-qc, (128, 2)))  # [128, 256]
    mlen = np.ascontiguousarray((1.0 - qc)[:, None])  # [128, 1]

    bo4 = np.ascontiguousarray(bo * 0.25)
    in_maps = []
    for c in range(N_CORES):
        b, g = c // 4, c % 4
        hsl = slice(g * FPC, (g + 1) * FPC)
        in_maps.append({
            "qkvT": np.ascontiguousarray(qkv[b].T.astype(bf)),
            "wq": np.ascontiguousarray(Wq[hsl, :].T.astype(bf)),
            "wk": np.ascontiguousarray(Wk[hsl, :].T.astype(bf)),
            "wv": np.ascontiguousarray(Wv[hsl, :].T.astype(bf)),
            "wo": np.ascontiguousarray(Wo[:, hsl].T.astype(bf)),
            "bq": np.ascontiguousarray(bq[hsl]),
            "bk": np.ascontiguousarray(bk[hsl]),
            "bv": np.ascontiguousarray(bv[hsl][None, :]),
            "bo": bo4,
            "cos2": cos2,
            "sinx": sinx,
            "mofs": mofs,
            "mlen": mlen,
        })

    trace = bool(os.environ.get("KERNEL_TRACE"))
    res = run_bass_kernel_spmd(nc, in_maps, list(range(N_CORES)), trace=trace)
    last_run_info["exec_time_ns"] = res.exec_time_ns
    last_run_info["results"] = res

    out = np.empty((B, S, D), dtype=np.float32)
    for b in range(B):
        oT = (res.results[4 * b]["out"].astype(np.float32)
              + res.results[4 * b + 1]["out"].astype(np.float32)
              + res.results[4 * b + 2]["out"].astype(np.float32)
              + res.results[4 * b + 3]["out"].astype(np.float32))
        out[b] = oT.T
    return out


# revision 39
# speedup vs baseline: 1.1589x; 1.1589x over previous
"""Distributed causal RoPE attention for Trainium2 (8 NeuronCores).

Problem: nn_CausalRpeAttn — B=2, S=2048, D=1024, H=16, Dh=64, fp32.

Sharding (data + head parallel): core c handles batch c//4 and heads
4*(c%4) .. 4*(c%4)+3 (a 256-wide feature slice). Wq/Wk/Wv are split
column-wise (by output head group), Wo row-wise. Each core writes its
full [1024, 2048] (transposed) bf16 partial output projection (with
bo/4 pre-added); the host unshards by summing the 4 partials per batch
and transposing back. Attention itself is fully independent per
(batch, head), so the only cross-core combination is that final sum.

Performance structure (v3):
 - Everything bf16 on the wires; fp32 only in PSUM accumulation and the
   softmax denominator reciprocal.
 - The Scalar engine runs (almost) nothing but the softmax exp; it is
   the phase-B bottleneck, so attention q-tiles are INTERLEAVED into
   the projection phase: q0,k0 proj -> v(first quarter) -> qt0-pair0
   scores (exp stream starts ~30us in) -> more v -> q1,k1 -> etc.
 - q/k projections feature-major (moving 512); v projected
   POSITION-major on the PE (x chunk stationary, Wv moving) - no
   transposes; bv added during the DVE PSUM->SBUF evacuation against a
   pre-broadcast bias tile.
 - Scores transposed sT[k, q]; the two heads of a 128-feature block run
   concurrently on disjoint PE row groups into different PSUM banks.
   Causal-trimmed moving ranges everywhere.
 - Diagonal-block causal masking via DVE tensor_mask against
   host-provided column-index/partition-threshold constants (GpSimd
   would pay a ~5us ucode reconfig each time it alternates
   affine_select with partition_broadcast).
 - v carries an appended ones-row so PV emits the softmax denominator
   row; denominators are copied to SBUF, inverted with the single-slot
   reciprocal_approx_fast (~5x faster than the iterative divide),
   partition-broadcast on GpSimd (its only compute), and two DVE muls
   produce the bf16 Wo input. Wo runs one q-tile late to hide that
   chain; the final q-tile's output casts alternate DVE/ACT to shorten
   the tail.
 - Input DMAs are split small and issued from sync/scalar/gpsimd
   queues so the first matmul starts ~6us in; rope swap DMAs ride the
   sync queue to keep the Scalar queue free for exp.
"""

import os
import ml_dtypes
import numpy as np

B, S, D, H, DH = 2, 2048, 1024, 16, 64
N_CORES = 8
FPC = 256  # features per core (4 heads)
QT = 512
NQT = S // QT  # 4
NST = S // 512  # 4 s-tiles for projections

_cache = {}
last_run_info = {}


def _build():
    import concourse.bass as bass
    import concourse.mybir as mybir
    import concourse.tile as tile
    from concourse import bacc

    F32 = mybir.dt.float32
    BF16 = mybir.dt.bfloat16
    EXP = mybir.ActivationFunctionType.Exp
    IDENT = mybir.ActivationFunctionType.Identity

    nc = bacc.Bacc("TRN2", target_bir_lowering=False, debug=False,
                   num_devices=N_CORES)

    qkvT_e = nc.dram_tensor("qkvT", [D, S], BF16, kind="ExternalInput").ap()
    wq_e = nc.dram_tensor("wq", [D, FPC], BF16, kind="ExternalInput").ap()
    wk_e = nc.dram_tensor("wk", [D, FPC], BF16, kind="ExternalInput").ap()
    wv_e = nc.dram_tensor("wv", [D, FPC], BF16, kind="ExternalInput").ap()
    wo_e = nc.dram_tensor("wo", [FPC, D], BF16, kind="ExternalInput").ap()
    bq_e = nc.dram_tensor("bq", [FPC], F32, kind="ExternalInput").ap()
    bk_e = nc.dram_tensor("bk", [FPC], F32, kind="ExternalInput").ap()
    bv_e = nc.dram_tensor("bv", [1, FPC], F32, kind="ExternalInput").ap()
    bo_e = nc.dram_tensor("bo", [D], F32, kind="ExternalInput").ap()
    cos2_e = nc.dram_tensor("cos2", [128, S], BF16, kind="ExternalInput").ap()
    sinx_e = nc.dram_tensor("sinx", [128, S], BF16, kind="ExternalInput").ap()
    mofs_e = nc.dram_tensor("mofs", [128, FPC], F32, kind="ExternalInput").ap()
    mlen_e = nc.dram_tensor("mlen", [128, 1], F32, kind="ExternalInput").ap()
    out_e = nc.dram_tensor("out", [D, S], BF16, kind="ExternalOutput").ap()

    from contextlib import ExitStack
    with tile.TileContext(nc) as tc:
        with ExitStack() as ctx:
            ep = ctx.enter_context
            consts = ep(tc.tile_pool(name="consts", bufs=1))
            xin_pool = ep(tc.tile_pool(name="xin", bufs=1))
            rope_pool = ep(tc.tile_pool(name="rope", bufs=4))
            qb_pool = ep(tc.tile_pool(name="qb", bufs=2))
            qbs_pool = ep(tc.tile_pool(name="qbs", bufs=2))
            tmp_pool = ep(tc.tile_pool(name="tmp", bufs=2))
            vsb_pool = ep(tc.tile_pool(name="vsb", bufs=1))
            probs_pool = ep(tc.tile_pool(name="probs", bufs=4))
            woin_pool = ep(tc.tile_pool(name="woin", bufs=2))
            rec_pool = ep(tc.tile_pool(name="rec", bufs=2))
            rb_pool = ep(tc.tile_pool(name="rb", bufs=2))
            osb_pool = ep(tc.tile_pool(name="osb", bufs=3))
            # PSUM: sc 2x[128,1024] = 4 banks, ps 4x[128,512] = 4 banks.
            sc_pool = ep(tc.tile_pool(name="sc", bufs=2, space="PSUM"))
            ps_pool = ep(tc.tile_pool(name="ps", bufs=4, space="PSUM"))

            # ---- input DMAs, split fine and spread across issue queues ----
            wq_sb = consts.tile([128, 8, FPC], BF16, tag="wq")
            wk_sb = consts.tile([128, 8, FPC], BF16, tag="wk")
            wv_sb = consts.tile([128, 8, FPC], BF16, tag="wv")
            wq_r = wq_e.rearrange("(kt p) f -> p kt f", p=128)
            wk_r = wk_e.rearrange("(kt p) f -> p kt f", p=128)
            wv_r = wv_e.rearrange("(kt p) f -> p kt f", p=128)
            x_all = xin_pool.tile([128, 8, S], BF16, tag="x")

            # sync queue: wq interleaved with the first x quarter so the
            # q-projection starts streaming immediately; swap DMAs and
            # output DMAs ride this queue later (program order).
            for kt in range(8):
                nc.sync.dma_start(out=wq_sb[:, kt, :], in_=wq_r[:, kt, :])
                nc.sync.dma_start(out=x_all[:, kt, 0:512],
                                  in_=qkvT_e[kt * 128:(kt + 1) * 128, 0:512])
            for kt in range(8):
                nc.sync.dma_start(out=x_all[:, kt, 512:1024],
                                  in_=qkvT_e[kt * 128:(kt + 1) * 128,
                                             512:1024])

            # scalar queue: ONLY small/early DMAs — anything more clogs
            # the in-order Scalar queue and delays the exp stream (each
            # DMA issue occupies the queue ~0.6us and waits for slots).
            b_sbs = []
            for name, be in (("bq", bq_e), ("bk", bk_e)):
                t = consts.tile([128, 2], F32, tag=name, name=name)
                nc.scalar.dma_start(out=t[:],
                                    in_=be.rearrange("(t p) -> p t", p=128))
                b_sbs.append(t)
            bq_sb, bk_sb = b_sbs
            # only the h0 halves of the rope tables ride the scalar
            # queue (needed ~19us); h1 goes via gpsimd so the early rope
            # swap DMAs get scalar DMA-queue slots sooner.
            cos2_sb = consts.tile([128, S], BF16, tag="cos2")
            sinx_sb = consts.tile([128, S], BF16, tag="sinx")
            nc.scalar.dma_start(out=cos2_sb[:, 0:1024], in_=cos2_e[:, 0:1024])
            nc.scalar.dma_start(out=sinx_sb[:, 0:1024], in_=sinx_e[:, 0:1024])

            # gpsimd queue: bv + mask consts first, wv/wk (needed ~14us),
            # then x quarters 2,3 and the rest.
            bv_row = consts.tile([1, FPC], F32, tag="bvr")
            nc.gpsimd.dma_start(out=bv_row[:], in_=bv_e)
            nc.gpsimd.dma_start(out=cos2_sb[:, 1024:2048],
                                in_=cos2_e[:, 1024:2048])
            nc.gpsimd.dma_start(out=sinx_sb[:, 1024:2048],
                                in_=sinx_e[:, 1024:2048])
            mofs_sb = consts.tile([128, FPC], F32, tag="mofs")
            nc.gpsimd.dma_start(out=mofs_sb[:], in_=mofs_e)
            mlen_sb = consts.tile([128, 1], F32, tag="mlen")
            nc.gpsimd.dma_start(out=mlen_sb[:], in_=mlen_e)
            for kt in range(0, 8, 4):
                nc.gpsimd.dma_start(out=wv_sb[:, kt:kt + 4, :],
                                    in_=wv_r[:, kt:kt + 4, :])
            for kt in range(0, 8, 4):
                nc.gpsimd.dma_start(out=wk_sb[:, kt:kt + 4, :],
                                    in_=wk_r[:, kt:kt + 4, :])
            for kt in range(8):
                nc.gpsimd.dma_start(
                    out=x_all[:, kt, 1024:2048],
                    in_=qkvT_e[kt * 128:(kt + 1) * 128, 1024:2048])
            wo_sb = consts.tile([128, 2, D], BF16, tag="wo")
            wo_r = wo_e.rearrange("(pt p) f -> p pt f", p=128)
            for pt in range(2):
                nc.gpsimd.dma_start(out=wo_sb[:, pt, :], in_=wo_r[:, pt, :])
            bo_sb = consts.tile([128, 8], F32, tag="bo")
            nc.gpsimd.dma_start(out=bo_sb[:],
                                in_=bo_e.rearrange("(t p) -> p t", p=128))
            bv_bc = consts.tile([128, FPC], F32, tag="bvb")
            nc.gpsimd.partition_broadcast(bv_bc[:], bv_row[0:1, :])


            # v with ones row: [pos 128, 16 s-tiles, 4 heads, 64+1]
            v_sb = vsb_pool.tile([128, 16, 4, DH + 1], BF16, tag="v")
            nc.vector.memset(v_sb[:, :, :, DH:DH + 1], 1.0)

            # rope targets: [feat 128, S] per partition-tile, bf16
            qrot = [rope_pool.tile([128, S], BF16, tag="rope", name=f"qrot{i}")
                    for i in range(2)]
            krot = [rope_pool.tile([128, S], BF16, tag="rope", name=f"krot{i}")
                    for i in range(2)]

            def proj_st(qb, w_sb, b_sb, pt, st, on_act):
                ss = slice(st * 512, (st + 1) * 512)
                ps = ps_pool.tile([128, 512], F32, tag="ps", name="psp")
                for kt in range(8):
                    nc.tensor.matmul(
                        ps[:], w_sb[:, kt, pt * 128:(pt + 1) * 128],
                        x_all[:, kt, ss],
                        start=(kt == 0), stop=(kt == 7))
                if on_act:
                    nc.scalar.activation(out=qb[:, ss], in_=ps[:],
                                         func=IDENT,
                                         bias=b_sb[:, pt:pt + 1])
                else:
                    nc.vector.tensor_scalar_add(
                        out=qb[:, ss], in0=ps[:],
                        scalar1=b_sb[:, pt:pt + 1])

            def proj_block(w_sb, b_sb, pt, on_act):
                # q/k projection for one 128-feature block -> qb bf16
                qb = qb_pool.tile([128, S], BF16, tag="qb", name="qb")
                for st in range(NST):
                    proj_st(qb, w_sb, b_sb, pt, st, on_act)
                return qb

            def rope_span(qb, qbs, dst, lo, hi, early=False):
                # dst = qb*cos + swap32(qb)*sinx for one free-dim span.
                # The front ropes issue their swaps on the scalar queue
                # (idle before the exp stream starts); later ones ride
                # sync so they never delay an exp.
                eng = nc.scalar if early else nc.sync
                hs = slice(lo, hi)
                nc.vector.tensor_mul(out=dst[:, hs], in0=qb[:, hs],
                                     in1=cos2_sb[:, hs])
                for blk in (0, 1):
                    p0 = blk * 64
                    eng.dma_start(out=qbs[p0:p0 + 32, hs],
                                  in_=qb[p0 + 32:p0 + 64, hs])
                    eng.dma_start(out=qbs[p0 + 32:p0 + 64, hs],
                                  in_=qb[p0:p0 + 32, hs])
                tmp = tmp_pool.tile([128, hi - lo], BF16, tag="tmp",
                                    name="tmp")
                nc.vector.tensor_mul(out=tmp[:], in0=qbs[:, hs],
                                     in1=sinx_sb[:, hs])
                nc.vector.tensor_add(out=dst[:, hs], in0=dst[:, hs],
                                     in1=tmp[:])

            def rope_half(qb, qbs, dst, h, early=False):
                rope_span(qb, qbs, dst, h * 1024, (h + 1) * 1024, early)

            def v_block(sti):
                # v projected position-major: x chunk stationary, Wv moving
                pv_ps = ps_pool.tile([128, FPC], F32, tag="ps", name="pv_ps")
                for kt in range(8):
                    nc.tensor.matmul(
                        pv_ps[:], x_all[:, kt, sti * 128:(sti + 1) * 128],
                        wv_sb[:, kt, :],
                        start=(kt == 0), stop=(kt == 7))
                nc.vector.tensor_add(
                    out=v_sb[:, sti, :, 0:DH],
                    in0=pv_ps[:].rearrange("p (h d) -> p h d", h=4),
                    in1=bv_bc[:].rearrange("p (h d) -> p h d", h=4))

            woin = [woin_pool.tile([128, S], BF16, tag="woin",
                                   name=f"woin{i}") for i in range(2)]

            def scores(kt, qt, pair, qt0):
                ksl = slice(kt * 128, (kt + 1) * 128)
                off = max(0, kt * 128 - qt * 512)
                ps_s = sc_pool.tile([128, 1024], F32, tag="sc", name="ps_s")
                psv = ps_s[:].rearrange("p (h q) -> p h q", h=2)
                for h in (0, 1):
                    nc.tensor.matmul(
                        psv[:, h, off:512],
                        krot[pair][h * 64:(h + 1) * 64, ksl],
                        qrot[pair][h * 64:(h + 1) * 64, qt0 + off:qt0 + 512],
                        start=True, stop=True)
                pr = probs_pool.tile([128, 1024], BF16, tag="pr", name="pr")
                prv = pr[:].rearrange("p (h q) -> p h q", h=2)
                nc.scalar.activation(out=prv[:, :, off:512],
                                     in_=psv[:, :, off:512],
                                     func=EXP, scale=0.125)
                if kt * 128 >= qt * 512:
                    # zero probs above the diagonal of this 128-block:
                    # keep column qc (0..127 per head) iff qc >= partition;
                    # TENSOR_MASK: out = in0 if (in1 + imm2) < s0 else 0
                    from concourse.dve_ops import TENSOR_MASK
                    for h in (0, 1):
                        nc.vector._custom_dve(
                            TENSOR_MASK,
                            out=prv[:, h, off:off + 128],
                            in0=prv[:, h, off:off + 128],
                            in1=mofs_sb[:, 0:128],
                            s0=mlen_sb[:, 0:1],
                            imm2=0.0)
                return pr

            def pv(kt, pr, pv_a, pv_b, pair, nkt, qt):
                off = max(0, kt * 128 - qt * 512)
                prv = pr[:].rearrange("p (h q) -> p h q", h=2)
                nc.tensor.matmul(
                    pv_a[0:DH + 1, off:512], v_sb[:, kt, 2 * pair, :],
                    prv[:, 0, off:512],
                    start=(kt == 0), stop=(kt == nkt - 1))
                nc.tensor.matmul(
                    pv_b[0:DH + 1, off:512], v_sb[:, kt, 2 * pair + 1, :],
                    prv[:, 1, off:512],
                    start=(kt == 0), stop=(kt == nkt - 1))

            def attn_pair(qt, pair, fillers=()):
                # scores+pv pipeline for one (q-tile, head-pair). fillers
                # are zero-arg callables emitting independent PE work
                # (v blocks / next projection tiles), woven one per kt so
                # the PE stays busy while the exp stream catches up.
                fillers = list(fillers)
                qt0 = qt * 512
                qsl = slice(qt0, qt0 + 512)
                pv_a = ps_pool.tile([DH + 1, 512], F32, tag="ps",
                                    name="pv_a")
                pv_b = ps_pool.tile([DH + 1, 512], F32, tag="ps",
                                    name="pv_b")
                nkt = 4 * qt + 4
                pr_prev = scores(0, qt, pair, qt0)
                if fillers:
                    fillers.pop(0)()
                for kt in range(1, nkt):
                    pr_k = scores(kt, qt, pair, qt0)
                    if fillers:
                        fillers.pop(0)()
                    pv(kt - 1, pr_prev, pv_a, pv_b, pair, nkt, qt)
                    pr_prev = pr_k
                pv(nkt - 1, pr_prev, pv_a, pv_b, pair, nkt, qt)
                for f in fillers:
                    f()

                # denominator reciprocal + broadcast + normalize
                den = rec_pool.tile([1, 1024], F32, tag="den", name="den")
                nc.vector.tensor_copy(out=den[0:1, 0:512],
                                      in_=pv_a[DH:DH + 1, :])
                nc.vector.tensor_copy(out=den[0:1, 512:1024],
                                      in_=pv_b[DH:DH + 1, :])
                rec = rec_pool.tile([1, 1024], F32, tag="rec", name="rec")
                nc.vector.reciprocal_approx_fast(
                    out=rec[0:1, :], in_=den[0:1, :])
                rb = rb_pool.tile([128, 1024], F32, tag="rb", name="rb")
                nc.gpsimd.partition_broadcast(rb[:], rec[0:1, :])
                nc.vector.tensor_mul(out=woin[pair][0:64, qsl],
                                     in0=pv_a[0:DH, :],
                                     in1=rb[0:64, 0:512])
                nc.vector.tensor_mul(out=woin[pair][64:128, qsl],
                                     in0=pv_b[0:DH, :],
                                     in1=rb[64:128, 512:1024])

            def wo_block(qt, last=False):
                qsl = slice(qt * 512, (qt + 1) * 512)
                for dm in range(8):
                    ps_o = ps_pool.tile([128, 512], F32, tag="ps",
                                        name="ps_o")
                    for pt in range(2):
                        nc.tensor.matmul(
                            ps_o[:], wo_sb[:, pt, dm * 128:(dm + 1) * 128],
                            woin[pt][:, qsl], start=(pt == 0), stop=(pt == 1))
                    ot = osb_pool.tile([128, QT], BF16, tag="ot", name="ot")
                    if last and dm % 2 == 1:
                        nc.scalar.activation(out=ot[:], in_=ps_o[:],
                                             func=IDENT,
                                             bias=bo_sb[:, dm:dm + 1])
                    else:
                        nc.vector.tensor_scalar_add(
                            out=ot[:], in0=ps_o[:],
                            scalar1=bo_sb[:, dm:dm + 1])
                    if last:
                        q0_ = qt * 512
                        nc.sync.dma_start(
                            out=out_e[dm * 128:(dm + 1) * 128,
                                      q0_:q0_ + 256], in_=ot[:, 0:256])
                        nc.sync.dma_start(
                            out=out_e[dm * 128:(dm + 1) * 128,
                                      q0_ + 256:q0_ + 512], in_=ot[:, 256:512])
                    else:
                        nc.sync.dma_start(
                            out=out_e[dm * 128:(dm + 1) * 128, qsl],
                            in_=ot[:])

            # ---- interleaved schedule ----
            # Fine-grained front: the first position-halves of the q0/k0
            # projections and their RoPE go first so qt0-pair0 scores (and
            # the exp stream) start ~20us in. Everything else — remaining
            # projection tiles, RoPE halves, v blocks — is woven into the
            # pair-0 attention as PE filler. Then pair 1 of every q-tile
            # (order 3,0,1,2) with the Wo blocks one step behind.
            qbq0 = qb_pool.tile([128, S], BF16, tag="qb", name="qb_q0")
            qbk0 = qb_pool.tile([128, S], BF16, tag="qb", name="qb_k0")
            qsq0 = qbs_pool.tile([128, S], BF16, tag="qbs", name="qs_q0")
            qsk0 = qbs_pool.tile([128, S], BF16, tag="qbs", name="qs_k0")
            qbq1 = qb_pool.tile([128, S], BF16, tag="qbx", name="qb_q1")
            qbk1 = qb_pool.tile([128, S], BF16, tag="qbx", name="qb_k1")
            qsq1 = qbs_pool.tile([128, S], BF16, tag="qbsx", name="qs_q1")
            qsk1 = qbs_pool.tile([128, S], BF16, tag="qbsx", name="qs_k1")

            proj_st(qbq0, wq_sb, bq_sb, 0, 0, False)
            rope_span(qbq0, qsq0, qrot[0], 0, 512, early=True)
            v_block(0)
            v_block(1)
            proj_st(qbq0, wq_sb, bq_sb, 0, 1, False)
            rope_span(qbq0, qsq0, qrot[0], 512, 1024, early=True)
            v_block(2)
            v_block(3)
            proj_st(qbk0, wk_sb, bk_sb, 0, 0, False)
            rope_span(qbk0, qsk0, krot[0], 0, 512, early=True)
            v_block(4)
            v_block(5)
            proj_st(qbk0, wk_sb, bk_sb, 0, 1, False)
            rope_span(qbk0, qsk0, krot[0], 512, 1024, early=True)
            v_block(6)
            v_block(7)

            fill = [
                # qt0 pair0 (4 kts)
                lambda: proj_st(qbq0, wq_sb, bq_sb, 0, 2, True),
                lambda: proj_st(qbk0, wk_sb, bk_sb, 0, 2, True),
                lambda: v_block(8),
                lambda: v_block(9),
                # qt1 pair0 (8 kts)
                lambda: proj_st(qbq0, wq_sb, bq_sb, 0, 3, True),
                lambda: proj_st(qbk0, wk_sb, bk_sb, 0, 3, True),
                lambda: rope_half(qbq0, qsq0, qrot[0], 1),
                lambda: rope_half(qbk0, qsk0, krot[0], 1),
                lambda: v_block(10),
                lambda: v_block(11),
                # qt2 pair0 (12 kts): v12..15 first (qt3's pv needs them)
                lambda: v_block(12),
                lambda: v_block(13),
                lambda: v_block(14),
                lambda: v_block(15),
                lambda: proj_st(qbq1, wq_sb, bq_sb, 1, 0, False),
                lambda: proj_st(qbq1, wq_sb, bq_sb, 1, 1, False),
                lambda: rope_half(qbq1, qsq1, qrot[1], 0),
                lambda: proj_st(qbq1, wq_sb, bq_sb, 1, 2, False),
                lambda: proj_st(qbq1, wq_sb, bq_sb, 1, 3, False),
                lambda: rope_half(qbq1, qsq1, qrot[1], 1),
                # qt3 pair0 (16 kts): k1 h0 only — k1 h1 moves into the
                # pair-1 phase where the PE otherwise idles
                lambda: proj_st(qbk1, wk_sb, bk_sb, 1, 0, False),
                lambda: proj_st(qbk1, wk_sb, bk_sb, 1, 1, False),
                lambda: rope_half(qbk1, qsk1, krot[1], 0),
            ]
            attn_pair(0, 0, fillers=fill[0:4])
            attn_pair(1, 0, fillers=fill[4:10])
            attn_pair(2, 0, fillers=fill[10:20])
            attn_pair(3, 0, fillers=fill[20:])

            fill1 = [
                lambda: proj_st(qbk1, wk_sb, bk_sb, 1, 2, False),
                lambda: proj_st(qbk1, wk_sb, bk_sb, 1, 3, False),
                lambda: rope_half(qbk1, qsk1, krot[1], 1),
            ]

            attn_pair(0, 1, fillers=fill1)
            attn_pair(3, 1)
            wo_block(0)
            attn_pair(2, 1)
            wo_block(3)
            attn_pair(1, 1)
            wo_block(2)
            wo_block(1, last=True)

    nc.compile()
    return nc


def kernel(qkv, cos, sin, Wq, bq, Wk, bk, Wv, bv, Wo, bo):
    from concourse.bass_utils import run_bass_kernel_spmd

    qkv = np.asarray(qkv, dtype=np.float32)
    cos = np.asarray(cos, dtype=np.float32)
    sin = np.asarray(sin, dtype=np.float32)
    Wq, bq = np.asarray(Wq, np.float32), np.asarray(bq, np.float32)
    Wk, bk = np.asarray(Wk, np.float32), np.asarray(bk, np.float32)
    Wv, bv = np.asarray(Wv, np.float32), np.asarray(bv, np.float32)
    Wo, bo = np.asarray(Wo, np.float32), np.asarray(bo, np.float32)

    if "nc" not in _cache:
        _cache["nc"] = _build()
    nc = _cache["nc"]

    bf = ml_dtypes.bfloat16
    cos2 = np.ascontiguousarray(np.tile(cos.T, (2, 1)).astype(bf))  # [128, S]
    sinx = np.tile(sin.T, (2, 1))
    sinx[0:32] *= -1.0
    sinx[64:96] *= -1.0
    sinx = np.ascontiguousarray(sinx.astype(bf))

    # tensor_mask consts: keep col qc (per 128-head-block) iff qc >= p
    #   keep iff mofs[p, qc] + 0 < mlen[p]  with mofs = -qc, mlen = 1-p
    qc = np.arange(128, dtype=np.float32)
    mofs = np.ascontiguousarray(np.tile(-qc, (128, 2)))  # [128, 256]
    mlen = np.ascontiguousarray((1.0 - qc)[:, None])  # [128, 1]

    bo4 = np.ascontiguousarray(bo * 0.25)
    in_maps = []
    for c in range(N_CORES):
        b, g = c // 4, c % 4
        hsl = slice(g * FPC, (g + 1) * FPC)
        in_maps.append({
            "qkvT": np.ascontiguousarray(qkv[b].T.astype(bf)),
            "wq": np.ascontiguousarray(Wq[hsl, :].T.astype(bf)),
            "wk": np.ascontiguousarray(Wk[hsl, :].T.astype(bf)),
            "wv": np.ascontiguousarray(Wv[hsl, :].T.astype(bf)),
            "wo": np.ascontiguousarray(Wo[:, hsl].T.astype(bf)),
            "bq": np.ascontiguousarray(bq[hsl]),
            "bk": np.ascontiguousarray(bk[hsl]),
            "bv": np.ascontiguousarray(bv[hsl][None, :]),
            "bo": bo4,
            "cos2": cos2,
            "sinx": sinx,
            "mofs": mofs,
            "mlen": mlen,
        })

    trace = bool(os.environ.get("KERNEL_TRACE"))
    res = run_bass_kernel_spmd(nc, in_maps, list(range(N_CORES)), trace=trace)
    last_run_info["exec_time_ns"] = res.exec_time_ns
    last_run_info["results"] = res

    out = np.empty((B, S, D), dtype=np.float32)
    for b in range(B):
        oT = (res.results[4 * b]["out"].astype(np.float32)
              + res.results[4 * b + 1]["out"].astype(np.float32)
              + res.results[4 * b + 2]["out"].astype(np.float32)
              + res.results[4 * b + 3]["out"].astype(np.float32))
        out[b] = oT.T
    return out


# revision 40
# speedup vs baseline: 1.1884x; 1.0255x over previous
"""Distributed causal RoPE attention for Trainium2 (8 NeuronCores).

Problem: nn_CausalRpeAttn — B=2, S=2048, D=1024, H=16, Dh=64, fp32.

Sharding (data + head parallel): core c handles batch c//4 and heads
4*(c%4) .. 4*(c%4)+3 (a 256-wide feature slice). Wq/Wk/Wv are split
column-wise (by output head group), Wo row-wise. Each core writes its
full [1024, 2048] (transposed) bf16 partial output projection (with
bo/4 pre-added); the host unshards by summing the 4 partials per batch
and transposing back. Attention itself is fully independent per
(batch, head), so the only cross-core combination is that final sum.

Performance structure (v3):
 - Everything bf16 on the wires; fp32 only in PSUM accumulation and the
   softmax denominator reciprocal.
 - The Scalar engine runs (almost) nothing but the softmax exp; it is
   the phase-B bottleneck, so attention q-tiles are INTERLEAVED into
   the projection phase: q0,k0 proj -> v(first quarter) -> qt0-pair0
   scores (exp stream starts ~30us in) -> more v -> q1,k1 -> etc.
 - q/k projections feature-major (moving 512); v projected
   POSITION-major on the PE (x chunk stationary, Wv moving) - no
   transposes; bv added during the DVE PSUM->SBUF evacuation against a
   pre-broadcast bias tile.
 - Scores transposed sT[k, q]; the two heads of a 128-feature block run
   concurrently on disjoint PE row groups into different PSUM banks.
   Causal-trimmed moving ranges everywhere.
 - Diagonal-block causal masking via DVE tensor_mask against
   host-provided column-index/partition-threshold constants (GpSimd
   would pay a ~5us ucode reconfig each time it alternates
   affine_select with partition_broadcast).
 - v carries an appended ones-row so PV emits the softmax denominator
   row; denominators are copied to SBUF, inverted with the single-slot
   reciprocal_approx_fast (~5x faster than the iterative divide),
   partition-broadcast on GpSimd (its only compute), and two DVE muls
   produce the bf16 Wo input. Wo runs one q-tile late to hide that
   chain; the final q-tile's output casts alternate DVE/ACT to shorten
   the tail.
 - Input DMAs are split small and issued from sync/scalar/gpsimd
   queues so the first matmul starts ~6us in; rope swap DMAs ride the
   sync queue to keep the Scalar queue free for exp.
"""

import os
import ml_dtypes
import numpy as np

B, S, D, H, DH = 2, 2048, 1024, 16, 64
N_CORES = 8
FPC = 256  # features per core (4 heads)
QT = 512
NQT = S // QT  # 4
NST = S // 512  # 4 s-tiles for projections

_cache = {}
last_run_info = {}


def _build():
    import concourse.bass as bass
    import concourse.mybir as mybir
    import concourse.tile as tile
    from concourse import bacc

    F32 = mybir.dt.float32
    BF16 = mybir.dt.bfloat16
    EXP = mybir.ActivationFunctionType.Exp
    IDENT = mybir.ActivationFunctionType.Identity

    nc = bacc.Bacc("TRN2", target_bir_lowering=False, debug=False,
                   num_devices=N_CORES)

    qkvT_e = nc.dram_tensor("qkvT", [D, S], BF16, kind="ExternalInput").ap()
    wq_e = nc.dram_tensor("wq", [D, FPC], BF16, kind="ExternalInput").ap()
    wk_e = nc.dram_tensor("wk", [D, FPC], BF16, kind="ExternalInput").ap()
    wv_e = nc.dram_tensor("wv", [D, FPC], BF16, kind="ExternalInput").ap()
    wo_e = nc.dram_tensor("wo", [FPC, D], BF16, kind="ExternalInput").ap()
    bq_e = nc.dram_tensor("bq", [FPC], F32, kind="ExternalInput").ap()
    bk_e = nc.dram_tensor("bk", [FPC], F32, kind="ExternalInput").ap()
    bv_e = nc.dram_tensor("bv", [1, FPC], F32, kind="ExternalInput").ap()
    bo_e = nc.dram_tensor("bo", [D], F32, kind="ExternalInput").ap()
    cos2_e = nc.dram_tensor("cos2", [128, S], BF16, kind="ExternalInput").ap()
    sinx_e = nc.dram_tensor("sinx", [128, S], BF16, kind="ExternalInput").ap()
    mofs_e = nc.dram_tensor("mofs", [128, FPC], F32, kind="ExternalInput").ap()
    mlen_e = nc.dram_tensor("mlen", [128, 1], F32, kind="ExternalInput").ap()
    out_e = nc.dram_tensor("out", [D, S], BF16, kind="ExternalOutput").ap()

    from contextlib import ExitStack
    with tile.TileContext(nc) as tc:
        with ExitStack() as ctx:
            ep = ctx.enter_context
            consts = ep(tc.tile_pool(name="consts", bufs=1))
            xin_pool = ep(tc.tile_pool(name="xin", bufs=1))
            rope_pool = ep(tc.tile_pool(name="rope", bufs=4))
            qb_pool = ep(tc.tile_pool(name="qb", bufs=2))
            qbs_pool = ep(tc.tile_pool(name="qbs", bufs=2))
            tmp_pool = ep(tc.tile_pool(name="tmp", bufs=2))
            vsb_pool = ep(tc.tile_pool(name="vsb", bufs=1))
            probs_pool = ep(tc.tile_pool(name="probs", bufs=4))
            woin_pool = ep(tc.tile_pool(name="woin", bufs=2))
            rec_pool = ep(tc.tile_pool(name="rec", bufs=2))
            rb_pool = ep(tc.tile_pool(name="rb", bufs=2))
            osb_pool = ep(tc.tile_pool(name="osb", bufs=3))
            # PSUM: sc 2x[128,1024] = 4 banks, ps 4x[128,512] = 4 banks.
            sc_pool = ep(tc.tile_pool(name="sc", bufs=2, space="PSUM"))
            ps_pool = ep(tc.tile_pool(name="ps", bufs=4, space="PSUM"))

            # ---- input DMAs, split fine and spread across issue queues ----
            wq_sb = consts.tile([128, 8, FPC], BF16, tag="wq")
            wk_sb = consts.tile([128, 8, FPC], BF16, tag="wk")
            wv_sb = consts.tile([128, 8, FPC], BF16, tag="wv")
            wq_r = wq_e.rearrange("(kt p) f -> p kt f", p=128)
            wk_r = wk_e.rearrange("(kt p) f -> p kt f", p=128)
            wv_r = wv_e.rearrange("(kt p) f -> p kt f", p=128)
            x_all = xin_pool.tile([128, 8, S], BF16, tag="x")

            # sync queue: wq interleaved with the first x quarter so the
            # q-projection starts streaming immediately; swap DMAs and
            # output DMAs ride this queue later (program order).
            for kt in range(8):
                nc.sync.dma_start(out=wq_sb[:, kt, :], in_=wq_r[:, kt, :])
                nc.sync.dma_start(out=x_all[:, kt, 0:512],
                                  in_=qkvT_e[kt * 128:(kt + 1) * 128, 0:512])
            for kt in range(8):
                nc.sync.dma_start(out=x_all[:, kt, 512:1024],
                                  in_=qkvT_e[kt * 128:(kt + 1) * 128,
                                             512:1024])

            # scalar queue: ONLY small/early DMAs — anything more clogs
            # the in-order Scalar queue and delays the exp stream (each
            # DMA issue occupies the queue ~0.6us and waits for slots).
            b_sbs = []
            for name, be in (("bq", bq_e), ("bk", bk_e)):
                t = consts.tile([128, 2], F32, tag=name, name=name)
                nc.scalar.dma_start(out=t[:],
                                    in_=be.rearrange("(t p) -> p t", p=128))
                b_sbs.append(t)
            bq_sb, bk_sb = b_sbs
            # only the h0 halves of the rope tables ride the scalar
            # queue (needed ~19us); h1 goes via gpsimd so the early rope
            # swap DMAs get scalar DMA-queue slots sooner.
            cos2_sb = consts.tile([128, S], BF16, tag="cos2")
            sinx_sb = consts.tile([128, S], BF16, tag="sinx")
            nc.scalar.dma_start(out=cos2_sb[:, 0:1024], in_=cos2_e[:, 0:1024])
            nc.scalar.dma_start(out=sinx_sb[:, 0:1024], in_=sinx_e[:, 0:1024])

            # gpsimd queue: bv + mask consts first, wv/wk (needed ~14us),
            # then x quarters 2,3 and the rest.
            bv_row = consts.tile([1, FPC], F32, tag="bvr")
            nc.gpsimd.dma_start(out=bv_row[:], in_=bv_e)
            nc.gpsimd.dma_start(out=cos2_sb[:, 1024:2048],
                                in_=cos2_e[:, 1024:2048])
            nc.gpsimd.dma_start(out=sinx_sb[:, 1024:2048],
                                in_=sinx_e[:, 1024:2048])
            mofs_sb = consts.tile([128, FPC], F32, tag="mofs")
            nc.gpsimd.dma_start(out=mofs_sb[:], in_=mofs_e)
            mlen_sb = consts.tile([128, 1], F32, tag="mlen")
            nc.gpsimd.dma_start(out=mlen_sb[:], in_=mlen_e)
            for kt in range(0, 8, 4):
                nc.gpsimd.dma_start(out=wv_sb[:, kt:kt + 4, :],
                                    in_=wv_r[:, kt:kt + 4, :])
            for kt in range(0, 8, 4):
                nc.gpsimd.dma_start(out=wk_sb[:, kt:kt + 4, :],
                                    in_=wk_r[:, kt:kt + 4, :])
            for kt in range(8):
                nc.gpsimd.dma_start(
                    out=x_all[:, kt, 1024:2048],
                    in_=qkvT_e[kt * 128:(kt + 1) * 128, 1024:2048])
            wo_sb = consts.tile([128, 2, D], BF16, tag="wo")
            wo_r = wo_e.rearrange("(pt p) f -> p pt f", p=128)
            for pt in range(2):
                nc.gpsimd.dma_start(out=wo_sb[:, pt, :], in_=wo_r[:, pt, :])
            bo_sb = consts.tile([128, 8], F32, tag="bo")
            nc.gpsimd.dma_start(out=bo_sb[:],
                                in_=bo_e.rearrange("(t p) -> p t", p=128))
            bv_bc = consts.tile([128, FPC], F32, tag="bvb")
            nc.gpsimd.partition_broadcast(bv_bc[:], bv_row[0:1, :])


            # v with ones row: [pos 128, 16 s-tiles, 4 heads, 64+1]
            v_sb = vsb_pool.tile([128, 16, 4, DH + 1], BF16, tag="v")
            nc.vector.memset(v_sb[:, :, :, DH:DH + 1], 1.0)

            # rope targets: [feat 128, S] per partition-tile, bf16
            qrot = [rope_pool.tile([128, S], BF16, tag="rope", name=f"qrot{i}")
                    for i in range(2)]
            krot = [rope_pool.tile([128, S], BF16, tag="rope", name=f"krot{i}")
                    for i in range(2)]

            def proj_st(qb, w_sb, b_sb, pt, st, on_act):
                ss = slice(st * 512, (st + 1) * 512)
                ps = ps_pool.tile([128, 512], F32, tag="ps", name="psp")
                for kt in range(8):
                    nc.tensor.matmul(
                        ps[:], w_sb[:, kt, pt * 128:(pt + 1) * 128],
                        x_all[:, kt, ss],
                        start=(kt == 0), stop=(kt == 7))
                if on_act:
                    nc.scalar.activation(out=qb[:, ss], in_=ps[:],
                                         func=IDENT,
                                         bias=b_sb[:, pt:pt + 1])
                else:
                    nc.vector.tensor_scalar_add(
                        out=qb[:, ss], in0=ps[:],
                        scalar1=b_sb[:, pt:pt + 1])

            def proj_block(w_sb, b_sb, pt, on_act):
                # q/k projection for one 128-feature block -> qb bf16
                qb = qb_pool.tile([128, S], BF16, tag="qb", name="qb")
                for st in range(NST):
                    proj_st(qb, w_sb, b_sb, pt, st, on_act)
                return qb

            def rope_span(qb, qbs, dst, lo, hi, early=False):
                # dst = qb*cos + swap32(qb)*sinx for one free-dim span.
                # The front ropes issue their swaps on the scalar queue
                # (idle before the exp stream starts); later ones ride
                # sync so they never delay an exp.
                eng = nc.scalar if early else nc.sync
                hs = slice(lo, hi)
                nc.vector.tensor_mul(out=dst[:, hs], in0=qb[:, hs],
                                     in1=cos2_sb[:, hs])
                for blk in (0, 1):
                    p0 = blk * 64
                    eng.dma_start(out=qbs[p0:p0 + 32, hs],
                                  in_=qb[p0 + 32:p0 + 64, hs])
                    eng.dma_start(out=qbs[p0 + 32:p0 + 64, hs],
                                  in_=qb[p0:p0 + 32, hs])
                tmp = tmp_pool.tile([128, hi - lo], BF16, tag="tmp",
                                    name="tmp")
                nc.vector.tensor_mul(out=tmp[:], in0=qbs[:, hs],
                                     in1=sinx_sb[:, hs])
                nc.vector.tensor_add(out=dst[:, hs], in0=dst[:, hs],
                                     in1=tmp[:])

            def rope_half(qb, qbs, dst, h, early=False):
                rope_span(qb, qbs, dst, h * 1024, (h + 1) * 1024, early)

            def v_block(sti):
                # v projected position-major: x chunk stationary, Wv moving
                pv_ps = ps_pool.tile([128, FPC], F32, tag="ps", name="pv_ps")
                for kt in range(8):
                    nc.tensor.matmul(
                        pv_ps[:], x_all[:, kt, sti * 128:(sti + 1) * 128],
                        wv_sb[:, kt, :],
                        start=(kt == 0), stop=(kt == 7))
                nc.vector.tensor_add(
                    out=v_sb[:, sti, :, 0:DH],
                    in0=pv_ps[:].rearrange("p (h d) -> p h d", h=4),
                    in1=bv_bc[:].rearrange("p (h d) -> p h d", h=4))

            woin = [woin_pool.tile([128, S], BF16, tag="woin",
                                   name=f"woin{i}") for i in range(2)]

            def scores(kt, qt, pair, qt0):
                ksl = slice(kt * 128, (kt + 1) * 128)
                off = max(0, kt * 128 - qt * 512)
                ps_s = sc_pool.tile([128, 1024], F32, tag="sc", name="ps_s")
                psv = ps_s[:].rearrange("p (h q) -> p h q", h=2)
                for h in (0, 1):
                    nc.tensor.matmul(
                        psv[:, h, off:512],
                        krot[pair][h * 64:(h + 1) * 64, ksl],
                        qrot[pair][h * 64:(h + 1) * 64, qt0 + off:qt0 + 512],
                        start=True, stop=True)
                pr = probs_pool.tile([128, 1024], BF16, tag="pr", name="pr")
                prv = pr[:].rearrange("p (h q) -> p h q", h=2)
                nc.scalar.activation(out=prv[:, :, off:512],
                                     in_=psv[:, :, off:512],
                                     func=EXP, scale=0.125)
                if kt * 128 >= qt * 512:
                    # zero probs above the diagonal of this 128-block:
                    # keep column qc (0..127 per head) iff qc >= partition;
                    # TENSOR_MASK: out = in0 if (in1 + imm2) < s0 else 0
                    from concourse.dve_ops import TENSOR_MASK
                    for h in (0, 1):
                        nc.vector._custom_dve(
                            TENSOR_MASK,
                            out=prv[:, h, off:off + 128],
                            in0=prv[:, h, off:off + 128],
                            in1=mofs_sb[:, 0:128],
                            s0=mlen_sb[:, 0:1],
                            imm2=0.0)
                return pr

            def pv(kt, pr, pv_a, pv_b, pair, nkt, qt):
                off = max(0, kt * 128 - qt * 512)
                prv = pr[:].rearrange("p (h q) -> p h q", h=2)
                nc.tensor.matmul(
                    pv_a[0:DH + 1, off:512], v_sb[:, kt, 2 * pair, :],
                    prv[:, 0, off:512],
                    start=(kt == 0), stop=(kt == nkt - 1))
                nc.tensor.matmul(
                    pv_b[0:DH + 1, off:512], v_sb[:, kt, 2 * pair + 1, :],
                    prv[:, 1, off:512],
                    start=(kt == 0), stop=(kt == nkt - 1))

            def attn_pair(qt, pair, fillers=()):
                # scores+pv pipeline for one (q-tile, head-pair). fillers
                # are zero-arg callables emitting independent PE work
                # (v blocks / next projection tiles), woven one per kt so
                # the PE stays busy while the exp stream catches up.
                fillers = list(fillers)
                qt0 = qt * 512
                qsl = slice(qt0, qt0 + 512)
                pv_a = ps_pool.tile([DH + 1, 512], F32, tag="ps",
                                    name="pv_a")
                pv_b = ps_pool.tile([DH + 1, 512], F32, tag="ps",
                                    name="pv_b")
                nkt = 4 * qt + 4
                pr_prev = scores(0, qt, pair, qt0)
                if fillers:
                    fillers.pop(0)()
                for kt in range(1, nkt):
                    pr_k = scores(kt, qt, pair, qt0)
                    if fillers:
                        fillers.pop(0)()
                    pv(kt - 1, pr_prev, pv_a, pv_b, pair, nkt, qt)
                    pr_prev = pr_k
                pv(nkt - 1, pr_prev, pv_a, pv_b, pair, nkt, qt)
                for f in fillers:
                    f()

                # denominator reciprocal + broadcast + normalize
                den = rec_pool.tile([1, 1024], F32, tag="den", name="den")
                nc.vector.tensor_copy(out=den[0:1, 0:512],
                                      in_=pv_a[DH:DH + 1, :])
                nc.vector.tensor_copy(out=den[0:1, 512:1024],
                                      in_=pv_b[DH:DH + 1, :])
                rec = rec_pool.tile([1, 1024], F32, tag="rec", name="rec")
                nc.vector.reciprocal_approx_fast(
                    out=rec[0:1, :], in_=den[0:1, :])
                rb = rb_pool.tile([128, 1024], F32, tag="rb", name="rb")
                nc.gpsimd.partition_broadcast(rb[:], rec[0:1, :])
                nc.vector.tensor_mul(out=woin[pair][0:64, qsl],
                                     in0=pv_a[0:DH, :],
                                     in1=rb[0:64, 0:512])
                nc.vector.tensor_mul(out=woin[pair][64:128, qsl],
                                     in0=pv_b[0:DH, :],
                                     in1=rb[64:128, 512:1024])

            def wo_block(qt, last=False):
                qsl = slice(qt * 512, (qt + 1) * 512)
                for dm in range(8):
                    ps_o = ps_pool.tile([128, 512], F32, tag="ps",
                                        name="ps_o")
                    for pt in range(2):
                        nc.tensor.matmul(
                            ps_o[:], wo_sb[:, pt, dm * 128:(dm + 1) * 128],
                            woin[pt][:, qsl], start=(pt == 0), stop=(pt == 1))
                    ot = osb_pool.tile([128, QT], BF16, tag="ot", name="ot")
                    if last and dm % 2 == 1:
                        nc.scalar.activation(out=ot[:], in_=ps_o[:],
                                             func=IDENT,
                                             bias=bo_sb[:, dm:dm + 1])
                    else:
                        nc.vector.tensor_scalar_add(
                            out=ot[:], in0=ps_o[:],
                            scalar1=bo_sb[:, dm:dm + 1])
                    if last:
                        q0_ = qt * 512
                        nc.sync.dma_start(
                            out=out_e[dm * 128:(dm + 1) * 128,
                                      q0_:q0_ + 256], in_=ot[:, 0:256])
                        nc.sync.dma_start(
                            out=out_e[dm * 128:(dm + 1) * 128,
                                      q0_ + 256:q0_ + 512], in_=ot[:, 256:512])
                    else:
                        nc.sync.dma_start(
                            out=out_e[dm * 128:(dm + 1) * 128, qsl],
                            in_=ot[:])

            # ---- interleaved schedule ----
            # Fine-grained front: the first position-halves of the q0/k0
            # projections and their RoPE go first so qt0-pair0 scores (and
            # the exp stream) start ~20us in. Everything else — remaining
            # projection tiles, RoPE halves, v blocks — is woven into the
            # pair-0 attention as PE filler. Then pair 1 of every q-tile
            # (order 3,0,1,2) with the Wo blocks one step behind.
            qbq0 = qb_pool.tile([128, S], BF16, tag="qb", name="qb_q0")
            qbk0 = qb_pool.tile([128, S], BF16, tag="qb", name="qb_k0")
            qsq0 = qbs_pool.tile([128, S], BF16, tag="qbs", name="qs_q0")
            qsk0 = qbs_pool.tile([128, S], BF16, tag="qbs", name="qs_k0")
            qbq1 = qb_pool.tile([128, S], BF16, tag="qbx", name="qb_q1")
            qbk1 = qb_pool.tile([128, S], BF16, tag="qbx", name="qb_k1")
            qsq1 = qbs_pool.tile([128, S], BF16, tag="qbsx", name="qs_q1")
            qsk1 = qbs_pool.tile([128, S], BF16, tag="qbsx", name="qs_k1")

            # Dummy matmuls on the first wq chunk fill the x-DMA wait
            # gaps so the HAM clock-gate stays open through the front
            # (otherwise the whole front runs at 1.2 GHz).
            warm_ps = ps_pool.tile([128, 512], F32, tag="ps", name="warm")
            warm_rhs = wq_sb[:, 0:2, :].rearrange("p a b -> p (a b)")

            def warm(n):
                for _ in range(n):
                    nc.tensor.matmul(warm_ps[:], wq_sb[:, 0, 0:128],
                                     warm_rhs[:, 0:512],
                                     start=True, stop=True)

            warm(4)
            proj_st(qbq0, wq_sb, bq_sb, 0, 0, False)
            rope_span(qbq0, qsq0, qrot[0], 0, 512, early=True)
            warm(3)
            v_block(0)
            warm(2)
            v_block(1)
            proj_st(qbq0, wq_sb, bq_sb, 0, 1, False)
            rope_span(qbq0, qsq0, qrot[0], 512, 1024, early=True)
            warm(2)
            v_block(2)
            v_block(3)
            proj_st(qbk0, wk_sb, bk_sb, 0, 0, False)
            rope_span(qbk0, qsk0, krot[0], 0, 512, early=True)
            warm(2)
            v_block(4)
            v_block(5)
            proj_st(qbk0, wk_sb, bk_sb, 0, 1, False)
            rope_span(qbk0, qsk0, krot[0], 512, 1024, early=True)
            v_block(6)
            v_block(7)

            fill = [
                # qt0 pair0 (4 kts)
                lambda: proj_st(qbq0, wq_sb, bq_sb, 0, 2, True),
                lambda: proj_st(qbk0, wk_sb, bk_sb, 0, 2, True),
                lambda: v_block(8),
                lambda: v_block(9),
                # qt1 pair0 (8 kts)
                lambda: proj_st(qbq0, wq_sb, bq_sb, 0, 3, True),
                lambda: proj_st(qbk0, wk_sb, bk_sb, 0, 3, True),
                lambda: rope_half(qbq0, qsq0, qrot[0], 1),
                lambda: rope_half(qbk0, qsk0, krot[0], 1),
                lambda: v_block(10),
                lambda: v_block(11),
                # qt2 pair0 (12 kts): v12..15 first (qt3's pv needs them)
                lambda: v_block(12),
                lambda: v_block(13),
                lambda: v_block(14),
                lambda: v_block(15),
                lambda: proj_st(qbq1, wq_sb, bq_sb, 1, 0, False),
                lambda: proj_st(qbq1, wq_sb, bq_sb, 1, 1, False),
                lambda: rope_half(qbq1, qsq1, qrot[1], 0),
                lambda: proj_st(qbq1, wq_sb, bq_sb, 1, 2, False),
                lambda: proj_st(qbq1, wq_sb, bq_sb, 1, 3, False),
                lambda: rope_half(qbq1, qsq1, qrot[1], 1),
                # qt3 pair0 (16 kts): k1 h0 only — k1 h1 moves into the
                # pair-1 phase where the PE otherwise idles
                lambda: proj_st(qbk1, wk_sb, bk_sb, 1, 0, False),
                lambda: proj_st(qbk1, wk_sb, bk_sb, 1, 1, False),
                lambda: rope_half(qbk1, qsk1, krot[1], 0),
            ]
            attn_pair(0, 0, fillers=fill[0:4])
            attn_pair(1, 0, fillers=fill[4:10])
            attn_pair(2, 0, fillers=fill[10:20])
            attn_pair(3, 0, fillers=fill[20:])

            fill1 = [
                lambda: proj_st(qbk1, wk_sb, bk_sb, 1, 2, False),
                lambda: proj_st(qbk1, wk_sb, bk_sb, 1, 3, False),
                lambda: rope_half(qbk1, qsk1, krot[1], 1),
            ]

            attn_pair(0, 1, fillers=fill1)
            attn_pair(3, 1)
            wo_block(0)
            attn_pair(2, 1)
            wo_block(3)
            attn_pair(1, 1)
            wo_block(2)
            wo_block(1, last=True)

    nc.compile()
    return nc


def kernel(qkv, cos, sin, Wq, bq, Wk, bk, Wv, bv, Wo, bo):
    from concourse.bass_utils import run_bass_kernel_spmd

    qkv = np.asarray(qkv, dtype=np.float32)
    cos = np.asarray(cos, dtype=np.float32)
    sin = np.asarray(sin, dtype=np.float32)
    Wq, bq = np.asarray(Wq, np.float32), np.asarray(bq, np.float32)
    Wk, bk = np.asarray(Wk, np.float32), np.asarray(bk, np.float32)
    Wv, bv = np.asarray(Wv, np.float32), np.asarray(bv, np.float32)
    Wo, bo = np.asarray(Wo, np.float32), np.asarray(bo, np.float32)

    if "nc" not in _cache:
        _cache["nc"] = _build()
    nc = _cache["nc"]

    bf = ml_dtypes.bfloat16
    cos2 = np.ascontiguousarray(np.tile(cos.T, (2, 1)).astype(bf))  # [128, S]
    sinx = np.tile(sin.T, (2, 1))
    sinx[0:32] *= -1.0
    sinx[64:96] *= -1.0
    sinx = np.ascontiguousarray(sinx.astype(bf))

    # tensor_mask consts: keep col qc (per 128-head-block) iff qc >= p
    #   keep iff mofs[p, qc] + 0 < mlen[p]  with mofs = -qc, mlen = 1-p
    qc = np.arange(128, dtype=np.float32)
    mofs = np.ascontiguousarray(np.tile(-qc, (128, 2)))  # [128, 256]
    mlen = np.ascontiguousarray((1.0 - qc)[:, None])  # [128, 1]

    bo4 = np.ascontiguousarray(bo * 0.25)
    in_maps = []
    for c in range(N_CORES):
        b, g = c // 4, c % 4
        hsl = slice(g * FPC, (g + 1) * FPC)
        in_maps.append({
            "qkvT": np.ascontiguousarray(qkv[b].T.astype(bf)),
            "wq": np.ascontiguousarray(Wq[hsl, :].T.astype(bf)),
            "wk": np.ascontiguousarray(Wk[hsl, :].T.astype(bf)),
            "wv": np.ascontiguousarray(Wv[hsl, :].T.astype(bf)),
            "wo": np.ascontiguousarray(Wo[:, hsl].T.astype(bf)),
            "bq": np.ascontiguousarray(bq[hsl]),
            "bk": np.ascontiguousarray(bk[hsl]),
            "bv": np.ascontiguousarray(bv[hsl][None, :]),
            "bo": bo4,
            "cos2": cos2,
            "sinx": sinx,
            "mofs": mofs,
            "mlen": mlen,
        })

    trace = bool(os.environ.get("KERNEL_TRACE"))
    res = run_bass_kernel_spmd(nc, in_maps, list(range(N_CORES)), trace=trace)
    last_run_info["exec_time_ns"] = res.exec_time_ns
    last_run_info["results"] = res

    out = np.empty((B, S, D), dtype=np.float32)
    for b in range(B):
        oT = (res.results[4 * b]["out"].astype(np.float32)
              + res.results[4 * b + 1]["out"].astype(np.float32)
              + res.results[4 * b + 2]["out"].astype(np.float32)
              + res.results[4 * b + 3]["out"].astype(np.float32))
        out[b] = oT.T
    return out


# revision 41
# speedup vs baseline: 1.1915x; 1.0025x over previous
"""Distributed causal RoPE attention for Trainium2 (8 NeuronCores).

Problem: nn_CausalRpeAttn — B=2, S=2048, D=1024, H=16, Dh=64, fp32.

Sharding (data + head parallel): core c handles batch c//4 and heads
4*(c%4) .. 4*(c%4)+3 (a 256-wide feature slice). Wq/Wk/Wv are split
column-wise (by output head group), Wo row-wise. Each core writes its
full [1024, 2048] (transposed) bf16 partial output projection (with
bo/4 pre-added); the host unshards by summing the 4 partials per batch
and transposing back. Attention itself is fully independent per
(batch, head), so the only cross-core combination is that final sum.

Performance structure (v3):
 - Everything bf16 on the wires; fp32 only in PSUM accumulation and the
   softmax denominator reciprocal.
 - The Scalar engine runs (almost) nothing but the softmax exp; it is
   the phase-B bottleneck, so attention q-tiles are INTERLEAVED into
   the projection phase: q0,k0 proj -> v(first quarter) -> qt0-pair0
   scores (exp stream starts ~30us in) -> more v -> q1,k1 -> etc.
 - q/k projections feature-major (moving 512); v projected
   POSITION-major on the PE (x chunk stationary, Wv moving) - no
   transposes; bv added during the DVE PSUM->SBUF evacuation against a
   pre-broadcast bias tile.
 - Scores transposed sT[k, q]; the two heads of a 128-feature block run
   concurrently on disjoint PE row groups into different PSUM banks.
   Causal-trimmed moving ranges everywhere.
 - Diagonal-block causal masking via DVE tensor_mask against
   host-provided column-index/partition-threshold constants (GpSimd
   would pay a ~5us ucode reconfig each time it alternates
   affine_select with partition_broadcast).
 - v carries an appended ones-row so PV emits the softmax denominator
   row; denominators are copied to SBUF, inverted with the single-slot
   reciprocal_approx_fast (~5x faster than the iterative divide),
   partition-broadcast on GpSimd (its only compute), and two DVE muls
   produce the bf16 Wo input. Wo runs one q-tile late to hide that
   chain; the final q-tile's output casts alternate DVE/ACT to shorten
   the tail.
 - Input DMAs are split small and issued from sync/scalar/gpsimd
   queues so the first matmul starts ~6us in; rope swap DMAs ride the
   sync queue to keep the Scalar queue free for exp.
"""

import os
import ml_dtypes
import numpy as np

B, S, D, H, DH = 2, 2048, 1024, 16, 64
N_CORES = 8
FPC = 256  # features per core (4 heads)
QT = 512
NQT = S // QT  # 4
NST = S // 512  # 4 s-tiles for projections

_cache = {}
last_run_info = {}


def _build():
    import concourse.bass as bass
    import concourse.mybir as mybir
    import concourse.tile as tile
    from concourse import bacc

    F32 = mybir.dt.float32
    BF16 = mybir.dt.bfloat16
    EXP = mybir.ActivationFunctionType.Exp
    IDENT = mybir.ActivationFunctionType.Identity

    nc = bacc.Bacc("TRN2", target_bir_lowering=False, debug=False,
                   num_devices=N_CORES)

    qkvT_e = nc.dram_tensor("qkvT", [D, S], BF16, kind="ExternalInput").ap()
    wq_e = nc.dram_tensor("wq", [D, FPC], BF16, kind="ExternalInput").ap()
    wk_e = nc.dram_tensor("wk", [D, FPC], BF16, kind="ExternalInput").ap()
    wv_e = nc.dram_tensor("wv", [D, FPC], BF16, kind="ExternalInput").ap()
    wo_e = nc.dram_tensor("wo", [FPC, D], BF16, kind="ExternalInput").ap()
    bq_e = nc.dram_tensor("bq", [FPC], F32, kind="ExternalInput").ap()
    bk_e = nc.dram_tensor("bk", [FPC], F32, kind="ExternalInput").ap()
    bv_e = nc.dram_tensor("bv", [1, FPC], F32, kind="ExternalInput").ap()
    bo_e = nc.dram_tensor("bo", [D], F32, kind="ExternalInput").ap()
    cos2_e = nc.dram_tensor("cos2", [128, S], BF16, kind="ExternalInput").ap()
    sinx_e = nc.dram_tensor("sinx", [128, S], BF16, kind="ExternalInput").ap()
    mofs_e = nc.dram_tensor("mofs", [128, FPC], F32, kind="ExternalInput").ap()
    mlen_e = nc.dram_tensor("mlen", [128, 1], F32, kind="ExternalInput").ap()
    out_e = nc.dram_tensor("out", [D, S], BF16, kind="ExternalOutput").ap()

    from contextlib import ExitStack
    with tile.TileContext(nc) as tc:
        with ExitStack() as ctx:
            ep = ctx.enter_context
            consts = ep(tc.tile_pool(name="consts", bufs=1))
            xin_pool = ep(tc.tile_pool(name="xin", bufs=1))
            rope_pool = ep(tc.tile_pool(name="rope", bufs=4))
            qb_pool = ep(tc.tile_pool(name="qb", bufs=2))
            qbs_pool = ep(tc.tile_pool(name="qbs", bufs=2))
            tmp_pool = ep(tc.tile_pool(name="tmp", bufs=2))
            vsb_pool = ep(tc.tile_pool(name="vsb", bufs=1))
            probs_pool = ep(tc.tile_pool(name="probs", bufs=4))
            woin_pool = ep(tc.tile_pool(name="woin", bufs=2))
            rec_pool = ep(tc.tile_pool(name="rec", bufs=2))
            rb_pool = ep(tc.tile_pool(name="rb", bufs=2))
            osb_pool = ep(tc.tile_pool(name="osb", bufs=3))
            # PSUM: sc 2x[128,1024] = 4 banks, ps 4x[128,512] = 4 banks.
            sc_pool = ep(tc.tile_pool(name="sc", bufs=2, space="PSUM"))
            ps_pool = ep(tc.tile_pool(name="ps", bufs=4, space="PSUM"))

            # ---- input DMAs, split fine and spread across issue queues ----
            wq_sb = consts.tile([128, 8, FPC], BF16, tag="wq")
            wk_sb = consts.tile([128, 8, FPC], BF16, tag="wk")
            wv_sb = consts.tile([128, 8, FPC], BF16, tag="wv")
            wq_r = wq_e.rearrange("(kt p) f -> p kt f", p=128)
            wk_r = wk_e.rearrange("(kt p) f -> p kt f", p=128)
            wv_r = wv_e.rearrange("(kt p) f -> p kt f", p=128)
            x_all = xin_pool.tile([128, 8, S], BF16, tag="x")

            # sync queue: wq interleaved with the first x quarter so the
            # q-projection starts streaming immediately; swap DMAs and
            # output DMAs ride this queue later (program order).
            for kt in range(8):
                nc.sync.dma_start(out=wq_sb[:, kt, :], in_=wq_r[:, kt, :])
                nc.sync.dma_start(out=x_all[:, kt, 0:512],
                                  in_=qkvT_e[kt * 128:(kt + 1) * 128, 0:512])
            for kt in range(8):
                nc.sync.dma_start(out=x_all[:, kt, 512:1024],
                                  in_=qkvT_e[kt * 128:(kt + 1) * 128,
                                             512:1024])

            # scalar queue: ONLY small/early DMAs — anything more clogs
            # the in-order Scalar queue and delays the exp stream (each
            # DMA issue occupies the queue ~0.6us and waits for slots).
            b_sbs = []
            for name, be in (("bq", bq_e), ("bk", bk_e)):
                t = consts.tile([128, 2], F32, tag=name, name=name)
                nc.scalar.dma_start(out=t[:],
                                    in_=be.rearrange("(t p) -> p t", p=128))
                b_sbs.append(t)
            bq_sb, bk_sb = b_sbs
            # only the h0 halves of the rope tables ride the scalar
            # queue (needed ~19us); h1 goes via gpsimd so the early rope
            # swap DMAs get scalar DMA-queue slots sooner.
            cos2_sb = consts.tile([128, S], BF16, tag="cos2")
            sinx_sb = consts.tile([128, S], BF16, tag="sinx")
            nc.scalar.dma_start(out=cos2_sb[:, 0:1024], in_=cos2_e[:, 0:1024])
            nc.scalar.dma_start(out=sinx_sb[:, 0:1024], in_=sinx_e[:, 0:1024])

            # gpsimd queue: bv + mask consts first, wv/wk (needed ~14us),
            # then x quarters 2,3 and the rest.
            bv_row = consts.tile([1, FPC], F32, tag="bvr")
            nc.gpsimd.dma_start(out=bv_row[:], in_=bv_e)
            nc.gpsimd.dma_start(out=cos2_sb[:, 1024:2048],
                                in_=cos2_e[:, 1024:2048])
            nc.gpsimd.dma_start(out=sinx_sb[:, 1024:2048],
                                in_=sinx_e[:, 1024:2048])
            mofs_sb = consts.tile([128, FPC], F32, tag="mofs")
            nc.gpsimd.dma_start(out=mofs_sb[:], in_=mofs_e)
            mlen_sb = consts.tile([128, 1], F32, tag="mlen")
            nc.gpsimd.dma_start(out=mlen_sb[:], in_=mlen_e)
            for kt in range(0, 8, 4):
                nc.gpsimd.dma_start(out=wv_sb[:, kt:kt + 4, :],
                                    in_=wv_r[:, kt:kt + 4, :])
            for kt in range(0, 8, 4):
                nc.gpsimd.dma_start(out=wk_sb[:, kt:kt + 4, :],
                                    in_=wk_r[:, kt:kt + 4, :])
            for kt in range(8):
                nc.gpsimd.dma_start(
                    out=x_all[:, kt, 1024:2048],
                    in_=qkvT_e[kt * 128:(kt + 1) * 128, 1024:2048])
            wo_sb = consts.tile([128, 2, D], BF16, tag="wo")
            wo_r = wo_e.rearrange("(pt p) f -> p pt f", p=128)
            for pt in range(2):
                nc.gpsimd.dma_start(out=wo_sb[:, pt, :], in_=wo_r[:, pt, :])
            bo_sb = consts.tile([128, 8], F32, tag="bo")
            nc.gpsimd.dma_start(out=bo_sb[:],
                                in_=bo_e.rearrange("(t p) -> p t", p=128))
            bv_bc = consts.tile([128, FPC], F32, tag="bvb")
            nc.gpsimd.partition_broadcast(bv_bc[:], bv_row[0:1, :])


            # v with ones row: [pos 128, 16 s-tiles, 4 heads, 64+1]
            v_sb = vsb_pool.tile([128, 16, 4, DH + 1], BF16, tag="v")
            nc.vector.memset(v_sb[:, :, :, DH:DH + 1], 1.0)

            # rope targets: [feat 128, S] per partition-tile, bf16
            qrot = [rope_pool.tile([128, S], BF16, tag="rope", name=f"qrot{i}")
                    for i in range(2)]
            krot = [rope_pool.tile([128, S], BF16, tag="rope", name=f"krot{i}")
                    for i in range(2)]

            def proj_st(qb, w_sb, b_sb, pt, st, on_act):
                ss = slice(st * 512, (st + 1) * 512)
                ps = ps_pool.tile([128, 512], F32, tag="ps", name="psp")
                for kt in range(8):
                    nc.tensor.matmul(
                        ps[:], w_sb[:, kt, pt * 128:(pt + 1) * 128],
                        x_all[:, kt, ss],
                        start=(kt == 0), stop=(kt == 7))
                if on_act:
                    nc.scalar.activation(out=qb[:, ss], in_=ps[:],
                                         func=IDENT,
                                         bias=b_sb[:, pt:pt + 1])
                else:
                    nc.vector.tensor_scalar_add(
                        out=qb[:, ss], in0=ps[:],
                        scalar1=b_sb[:, pt:pt + 1])

            def proj_block(w_sb, b_sb, pt, on_act):
                # q/k projection for one 128-feature block -> qb bf16
                qb = qb_pool.tile([128, S], BF16, tag="qb", name="qb")
                for st in range(NST):
                    proj_st(qb, w_sb, b_sb, pt, st, on_act)
                return qb

            def rope_span(qb, qbs, dst, lo, hi, early=False):
                # dst = qb*cos + swap32(qb)*sinx for one free-dim span.
                # The front ropes issue their swaps on the scalar queue
                # (idle before the exp stream starts); later ones ride
                # sync so they never delay an exp.
                eng = nc.scalar if early else nc.sync
                hs = slice(lo, hi)
                nc.vector.tensor_mul(out=dst[:, hs], in0=qb[:, hs],
                                     in1=cos2_sb[:, hs])
                for blk in (0, 1):
                    p0 = blk * 64
                    eng.dma_start(out=qbs[p0:p0 + 32, hs],
                                  in_=qb[p0 + 32:p0 + 64, hs])
                    eng.dma_start(out=qbs[p0 + 32:p0 + 64, hs],
                                  in_=qb[p0:p0 + 32, hs])
                tmp = tmp_pool.tile([128, hi - lo], BF16, tag="tmp",
                                    name="tmp")
                nc.vector.tensor_mul(out=tmp[:], in0=qbs[:, hs],
                                     in1=sinx_sb[:, hs])
                nc.vector.tensor_add(out=dst[:, hs], in0=dst[:, hs],
                                     in1=tmp[:])

            def rope_half(qb, qbs, dst, h, early=False):
                rope_span(qb, qbs, dst, h * 1024, (h + 1) * 1024, early)

            def v_block(sti):
                # v projected position-major: x chunk stationary, Wv moving
                pv_ps = ps_pool.tile([128, FPC], F32, tag="ps", name="pv_ps")
                for kt in range(8):
                    nc.tensor.matmul(
                        pv_ps[:], x_all[:, kt, sti * 128:(sti + 1) * 128],
                        wv_sb[:, kt, :],
                        start=(kt == 0), stop=(kt == 7))
                nc.vector.tensor_add(
                    out=v_sb[:, sti, :, 0:DH],
                    in0=pv_ps[:].rearrange("p (h d) -> p h d", h=4),
                    in1=bv_bc[:].rearrange("p (h d) -> p h d", h=4))

            woin = [woin_pool.tile([128, S], BF16, tag="woin",
                                   name=f"woin{i}") for i in range(2)]

            def scores(kt, qt, pair, qt0):
                ksl = slice(kt * 128, (kt + 1) * 128)
                off = max(0, kt * 128 - qt * 512)
                ps_s = sc_pool.tile([128, 1024], F32, tag="sc", name="ps_s")
                psv = ps_s[:].rearrange("p (h q) -> p h q", h=2)
                for h in (0, 1):
                    nc.tensor.matmul(
                        psv[:, h, off:512],
                        krot[pair][h * 64:(h + 1) * 64, ksl],
                        qrot[pair][h * 64:(h + 1) * 64, qt0 + off:qt0 + 512],
                        start=True, stop=True)
                pr = probs_pool.tile([128, 1024], BF16, tag="pr", name="pr")
                prv = pr[:].rearrange("p (h q) -> p h q", h=2)
                nc.scalar.activation(out=prv[:, :, off:512],
                                     in_=psv[:, :, off:512],
                                     func=EXP, scale=0.125)
                if kt * 128 >= qt * 512:
                    # zero probs above the diagonal of this 128-block:
                    # keep column qc (0..127 per head) iff qc >= partition;
                    # TENSOR_MASK: out = in0 if (in1 + imm2) < s0 else 0
                    from concourse.dve_ops import TENSOR_MASK
                    for h in (0, 1):
                        nc.vector._custom_dve(
                            TENSOR_MASK,
                            out=prv[:, h, off:off + 128],
                            in0=prv[:, h, off:off + 128],
                            in1=mofs_sb[:, 0:128],
                            s0=mlen_sb[:, 0:1],
                            imm2=0.0)
                return pr

            def pv(kt, pr, pv_a, pv_b, pair, nkt, qt):
                off = max(0, kt * 128 - qt * 512)
                prv = pr[:].rearrange("p (h q) -> p h q", h=2)
                nc.tensor.matmul(
                    pv_a[0:DH + 1, off:512], v_sb[:, kt, 2 * pair, :],
                    prv[:, 0, off:512],
                    start=(kt == 0), stop=(kt == nkt - 1))
                nc.tensor.matmul(
                    pv_b[0:DH + 1, off:512], v_sb[:, kt, 2 * pair + 1, :],
                    prv[:, 1, off:512],
                    start=(kt == 0), stop=(kt == nkt - 1))

            def attn_pair(qt, pair, fillers=()):
                # scores+pv pipeline for one (q-tile, head-pair). fillers
                # are zero-arg callables emitting independent PE work
                # (v blocks / next projection tiles), woven one per kt so
                # the PE stays busy while the exp stream catches up.
                fillers = list(fillers)
                qt0 = qt * 512
                qsl = slice(qt0, qt0 + 512)
                pv_a = ps_pool.tile([DH + 1, 512], F32, tag="ps",
                                    name="pv_a")
                pv_b = ps_pool.tile([DH + 1, 512], F32, tag="ps",
                                    name="pv_b")
                nkt = 4 * qt + 4
                pr_prev = scores(0, qt, pair, qt0)
                if fillers:
                    fillers.pop(0)()
                for kt in range(1, nkt):
                    pr_k = scores(kt, qt, pair, qt0)
                    if fillers:
                        fillers.pop(0)()
                    pv(kt - 1, pr_prev, pv_a, pv_b, pair, nkt, qt)
                    pr_prev = pr_k
                pv(nkt - 1, pr_prev, pv_a, pv_b, pair, nkt, qt)
                for f in fillers:
                    f()

                # denominator reciprocal + broadcast + normalize
                den = rec_pool.tile([1, 1024], F32, tag="den", name="den")
                nc.vector.tensor_copy(out=den[0:1, 0:512],
                                      in_=pv_a[DH:DH + 1, :])
                nc.vector.tensor_copy(out=den[0:1, 512:1024],
                                      in_=pv_b[DH:DH + 1, :])
                rec = rec_pool.tile([1, 1024], F32, tag="rec", name="rec")
                nc.vector.reciprocal_approx_fast(
                    out=rec[0:1, :], in_=den[0:1, :])
                rb = rb_pool.tile([128, 1024], F32, tag="rb", name="rb")
                nc.gpsimd.partition_broadcast(rb[:], rec[0:1, :])
                nc.vector.tensor_mul(out=woin[pair][0:64, qsl],
                                     in0=pv_a[0:DH, :],
                                     in1=rb[0:64, 0:512])
                nc.vector.tensor_mul(out=woin[pair][64:128, qsl],
                                     in0=pv_b[0:DH, :],
                                     in1=rb[64:128, 512:1024])

            def wo_block(qt, last=False):
                qsl = slice(qt * 512, (qt + 1) * 512)
                for dm in range(8):
                    ps_o = ps_pool.tile([128, 512], F32, tag="ps",
                                        name="ps_o")
                    for pt in range(2):
                        nc.tensor.matmul(
                            ps_o[:], wo_sb[:, pt, dm * 128:(dm + 1) * 128],
                            woin[pt][:, qsl], start=(pt == 0), stop=(pt == 1))
                    ot = osb_pool.tile([128, QT], BF16, tag="ot", name="ot")
                    if last and dm % 2 == 1:
                        nc.scalar.activation(out=ot[:], in_=ps_o[:],
                                             func=IDENT,
                                             bias=bo_sb[:, dm:dm + 1])
                    else:
                        nc.vector.tensor_scalar_add(
                            out=ot[:], in0=ps_o[:],
                            scalar1=bo_sb[:, dm:dm + 1])
                    if last:
                        q0_ = qt * 512
                        nc.sync.dma_start(
                            out=out_e[dm * 128:(dm + 1) * 128,
                                      q0_:q0_ + 256], in_=ot[:, 0:256])
                        nc.sync.dma_start(
                            out=out_e[dm * 128:(dm + 1) * 128,
                                      q0_ + 256:q0_ + 512], in_=ot[:, 256:512])
                    else:
                        nc.sync.dma_start(
                            out=out_e[dm * 128:(dm + 1) * 128, qsl],
                            in_=ot[:])

            # ---- interleaved schedule ----
            # Fine-grained front: the first position-halves of the q0/k0
            # projections and their RoPE go first so qt0-pair0 scores (and
            # the exp stream) start ~20us in. Everything else — remaining
            # projection tiles, RoPE halves, v blocks — is woven into the
            # pair-0 attention as PE filler. Then pair 1 of every q-tile
            # (order 3,0,1,2) with the Wo blocks one step behind.
            qbq0 = qb_pool.tile([128, S], BF16, tag="qb", name="qb_q0")
            qbk0 = qb_pool.tile([128, S], BF16, tag="qb", name="qb_k0")
            qsq0 = qbs_pool.tile([128, S], BF16, tag="qbs", name="qs_q0")
            qsk0 = qbs_pool.tile([128, S], BF16, tag="qbs", name="qs_k0")
            qbq1 = qb_pool.tile([128, S], BF16, tag="qbx", name="qb_q1")
            qbk1 = qb_pool.tile([128, S], BF16, tag="qbx", name="qb_k1")
            qsq1 = qbs_pool.tile([128, S], BF16, tag="qbsx", name="qs_q1")
            qsk1 = qbs_pool.tile([128, S], BF16, tag="qbsx", name="qs_k1")

            # Dummy matmuls on the first wq chunk fill the x-DMA wait
            # gaps so the HAM clock-gate stays open through the front
            # (otherwise the whole front runs at 1.2 GHz).
            warm_ps = ps_pool.tile([128, 512], F32, tag="ps", name="warm")
            warm_rhs = wq_sb[:, 0:2, :].rearrange("p a b -> p (a b)")

            def warm(n):
                for _ in range(n):
                    nc.tensor.matmul(warm_ps[:], wq_sb[:, 0, 0:128],
                                     warm_rhs[:, 0:512],
                                     start=True, stop=True)

            warm(4)
            proj_st(qbq0, wq_sb, bq_sb, 0, 0, False)
            rope_span(qbq0, qsq0, qrot[0], 0, 512, early=True)
            warm(3)
            v_block(0)
            warm(2)
            v_block(1)
            proj_st(qbq0, wq_sb, bq_sb, 0, 1, False)
            rope_span(qbq0, qsq0, qrot[0], 512, 1024, early=True)
            warm(2)
            v_block(2)
            warm(2)
            v_block(3)
            proj_st(qbk0, wk_sb, bk_sb, 0, 0, False)
            rope_span(qbk0, qsk0, krot[0], 0, 512, early=True)
            warm(2)
            v_block(4)
            warm(2)
            v_block(5)
            proj_st(qbk0, wk_sb, bk_sb, 0, 1, False)
            rope_span(qbk0, qsk0, krot[0], 512, 1024, early=True)
            warm(2)
            v_block(6)
            warm(2)
            v_block(7)

            fill = [
                # qt0 pair0 (4 kts)
                lambda: proj_st(qbq0, wq_sb, bq_sb, 0, 2, True),
                lambda: proj_st(qbk0, wk_sb, bk_sb, 0, 2, True),
                lambda: v_block(8),
                lambda: v_block(9),
                # qt1 pair0 (8 kts)
                lambda: proj_st(qbq0, wq_sb, bq_sb, 0, 3, True),
                lambda: proj_st(qbk0, wk_sb, bk_sb, 0, 3, True),
                lambda: rope_half(qbq0, qsq0, qrot[0], 1),
                lambda: rope_half(qbk0, qsk0, krot[0], 1),
                lambda: v_block(10),
                lambda: v_block(11),
                # qt2 pair0 (12 kts): v12..15 first (qt3's pv needs them)
                lambda: v_block(12),
                lambda: v_block(13),
                lambda: v_block(14),
                lambda: v_block(15),
                lambda: proj_st(qbq1, wq_sb, bq_sb, 1, 0, False),
                lambda: proj_st(qbq1, wq_sb, bq_sb, 1, 1, False),
                lambda: rope_half(qbq1, qsq1, qrot[1], 0),
                lambda: proj_st(qbq1, wq_sb, bq_sb, 1, 2, False),
                lambda: proj_st(qbq1, wq_sb, bq_sb, 1, 3, False),
                lambda: rope_half(qbq1, qsq1, qrot[1], 1),
                # qt3 pair0 (16 kts): k1 h0 only — k1 h1 moves into the
                # pair-1 phase where the PE otherwise idles
                lambda: proj_st(qbk1, wk_sb, bk_sb, 1, 0, False),
                lambda: proj_st(qbk1, wk_sb, bk_sb, 1, 1, False),
                lambda: rope_half(qbk1, qsk1, krot[1], 0),
            ]
            attn_pair(0, 0, fillers=fill[0:4])
            attn_pair(1, 0, fillers=fill[4:10])
            attn_pair(2, 0, fillers=fill[10:20])
            attn_pair(3, 0, fillers=fill[20:])

            fill1 = [
                lambda: proj_st(qbk1, wk_sb, bk_sb, 1, 2, False),
                lambda: proj_st(qbk1, wk_sb, bk_sb, 1, 3, False),
                lambda: rope_half(qbk1, qsk1, krot[1], 1),
            ]

            attn_pair(0, 1, fillers=fill1)
            attn_pair(3, 1)
            wo_block(0)
            attn_pair(2, 1)
            wo_block(3)
            attn_pair(1, 1)
            warm2_ps = ps_pool.tile([128, 512], F32, tag="ps", name="warm2")
            for _ in range(3):
                nc.tensor.matmul(warm2_ps[:], wq_sb[:, 0, 0:128],
                                 warm_rhs[:, 0:512], start=True, stop=True)
            wo_block(2)
            for _ in range(2):
                nc.tensor.matmul(warm2_ps[:], wq_sb[:, 0, 0:128],
                                 warm_rhs[:, 0:512], start=True, stop=True)
            wo_block(1, last=True)

    nc.compile()
    return nc


def kernel(qkv, cos, sin, Wq, bq, Wk, bk, Wv, bv, Wo, bo):
    from concourse.bass_utils import run_bass_kernel_spmd

    qkv = np.asarray(qkv, dtype=np.float32)
    cos = np.asarray(cos, dtype=np.float32)
    sin = np.asarray(sin, dtype=np.float32)
    Wq, bq = np.asarray(Wq, np.float32), np.asarray(bq, np.float32)
    Wk, bk = np.asarray(Wk, np.float32), np.asarray(bk, np.float32)
    Wv, bv = np.asarray(Wv, np.float32), np.asarray(bv, np.float32)
    Wo, bo = np.asarray(Wo, np.float32), np.asarray(bo, np.float32)

    if "nc" not in _cache:
        _cache["nc"] = _build()
    nc = _cache["nc"]

    bf = ml_dtypes.bfloat16
    cos2 = np.ascontiguousarray(np.tile(cos.T, (2, 1)).astype(bf))  # [128, S]
    sinx = np.tile(sin.T, (2, 1))
    sinx[0:32] *= -1.0
    sinx[64:96] *= -1.0
    sinx = np.ascontiguousarray(sinx.astype(bf))

    # tensor_mask consts: keep col qc (per 128-head-block) iff qc >= p
    #   keep iff mofs[p, qc] + 0 < mlen[p]  with mofs = -qc, mlen = 1-p
    qc = np.arange(128, dtype=np.float32)
    mofs = np.ascontiguousarray(np.tile(-qc, (128, 2)))  # [128, 256]
    mlen = np.ascontiguousarray((1.0 - qc)[:, None])  # [128, 1]

    bo4 = np.ascontiguousarray(bo * 0.25)
    in_maps = []
    for c in range(N_CORES):
        b, g = c // 4, c % 4
        hsl = slice(g * FPC, (g + 1) * FPC)
        in_maps.append({
            "qkvT": np.ascontiguousarray(qkv[b].T.astype(bf)),
            "wq": np.ascontiguousarray(Wq[hsl, :].T.astype(bf)),
            "wk": np.ascontiguousarray(Wk[hsl, :].T.astype(bf)),
            "wv": np.ascontiguousarray(Wv[hsl, :].T.astype(bf)),
            "wo": np.ascontiguousarray(Wo[:, hsl].T.astype(bf)),
            "bq": np.ascontiguousarray(bq[hsl]),
            "bk": np.ascontiguousarray(bk[hsl]),
            "bv": np.ascontiguousarray(bv[hsl][None, :]),
            "bo": bo4,
            "cos2": cos2,
            "sinx": sinx,
            "mofs": mofs,
            "mlen": mlen,
        })

    trace = bool(os.environ.get("KERNEL_TRACE"))
    res = run_bass_kernel_spmd(nc, in_maps, list(range(N_CORES)), trace=trace)
    last_run_info["exec_time_ns"] = res.exec_time_ns
    last_run_info["results"] = res

    out = np.empty((B, S, D), dtype=np.float32)
    for b in range(B):
        oT = (res.results[4 * b]["out"].astype(np.float32)
              + res.results[4 * b + 1]["out"].astype(np.float32)
              + res.results[4 * b + 2]["out"].astype(np.float32)
              + res.results[4 * b + 3]["out"].astype(np.float32))
        out[b] = oT.T
    return out


# revision 43
# speedup vs baseline: 1.2108x; 1.0163x over previous
"""Distributed causal RoPE attention for Trainium2 (8 NeuronCores).

Problem: nn_CausalRpeAttn — B=2, S=2048, D=1024, H=16, Dh=64, fp32.

Sharding (data + head parallel): core c handles batch c//4 and heads
4*(c%4) .. 4*(c%4)+3 (a 256-wide feature slice). Wq/Wk/Wv are split
column-wise (by output head group), Wo row-wise. Each core writes its
full [1024, 2048] (transposed) bf16 partial output projection (with
bo/4 pre-added); the host unshards by summing the 4 partials per batch
and transposing back. Attention itself is fully independent per
(batch, head), so the only cross-core combination is that final sum.

Performance structure (v3):
 - Everything bf16 on the wires; fp32 only in PSUM accumulation and the
   softmax denominator reciprocal.
 - The Scalar engine runs (almost) nothing but the softmax exp; it is
   the phase-B bottleneck, so attention q-tiles are INTERLEAVED into
   the projection phase: q0,k0 proj -> v(first quarter) -> qt0-pair0
   scores (exp stream starts ~30us in) -> more v -> q1,k1 -> etc.
 - q/k projections feature-major (moving 512); v projected
   POSITION-major on the PE (x chunk stationary, Wv moving) - no
   transposes; bv added during the DVE PSUM->SBUF evacuation against a
   pre-broadcast bias tile.
 - Scores transposed sT[k, q]; the two heads of a 128-feature block run
   concurrently on disjoint PE row groups into different PSUM banks.
   Causal-trimmed moving ranges everywhere.
 - Diagonal-block causal masking via DVE tensor_mask against
   host-provided column-index/partition-threshold constants (GpSimd
   would pay a ~5us ucode reconfig each time it alternates
   affine_select with partition_broadcast).
 - v carries an appended ones-row so PV emits the softmax denominator
   row; denominators are copied to SBUF, inverted with the single-slot
   reciprocal_approx_fast (~5x faster than the iterative divide),
   partition-broadcast on GpSimd (its only compute), and two DVE muls
   produce the bf16 Wo input. Wo runs one q-tile late to hide that
   chain; the final q-tile's output casts alternate DVE/ACT to shorten
   the tail.
 - Input DMAs are split small and issued from sync/scalar/gpsimd
   queues so the first matmul starts ~6us in; rope swap DMAs ride the
   sync queue to keep the Scalar queue free for exp.
"""

import os
import ml_dtypes
import numpy as np

B, S, D, H, DH = 2, 2048, 1024, 16, 64
N_CORES = 8
FPC = 256  # features per core (4 heads)
QT = 512
NQT = S // QT  # 4
NST = S // 512  # 4 s-tiles for projections

_cache = {}
last_run_info = {}


def _build():
    import concourse.bass as bass
    import concourse.mybir as mybir
    import concourse.tile as tile
    from concourse import bacc

    F32 = mybir.dt.float32
    BF16 = mybir.dt.bfloat16
    EXP = mybir.ActivationFunctionType.Exp
    IDENT = mybir.ActivationFunctionType.Identity

    nc = bacc.Bacc("TRN2", target_bir_lowering=False, debug=False,
                   num_devices=N_CORES)

    qkvT_e = nc.dram_tensor("qkvT", [D, S], BF16, kind="ExternalInput").ap()
    wq_e = nc.dram_tensor("wq", [D, FPC], BF16, kind="ExternalInput").ap()
    wk_e = nc.dram_tensor("wk", [D, FPC], BF16, kind="ExternalInput").ap()
    wv_e = nc.dram_tensor("wv", [D, FPC], BF16, kind="ExternalInput").ap()
    wo_e = nc.dram_tensor("wo", [FPC, D], BF16, kind="ExternalInput").ap()
    bq_e = nc.dram_tensor("bq", [FPC], F32, kind="ExternalInput").ap()
    bk_e = nc.dram_tensor("bk", [FPC], F32, kind="ExternalInput").ap()
    bv_e = nc.dram_tensor("bv", [1, FPC], F32, kind="ExternalInput").ap()
    bo_e = nc.dram_tensor("bo", [D], F32, kind="ExternalInput").ap()
    cos2_e = nc.dram_tensor("cos2", [128, S], BF16, kind="ExternalInput").ap()
    sinx_e = nc.dram_tensor("sinx", [128, S], BF16, kind="ExternalInput").ap()
    mofs_e = nc.dram_tensor("mofs", [128, FPC], F32, kind="ExternalInput").ap()
    mlen_e = nc.dram_tensor("mlen", [128, 1], F32, kind="ExternalInput").ap()
    out_e = nc.dram_tensor("out", [D, S], BF16, kind="ExternalOutput").ap()

    from contextlib import ExitStack
    with tile.TileContext(nc) as tc:
        with ExitStack() as ctx:
            ep = ctx.enter_context
            consts = ep(tc.tile_pool(name="consts", bufs=1))
            xin_pool = ep(tc.tile_pool(name="xin", bufs=1))
            rope_pool = ep(tc.tile_pool(name="rope", bufs=4))
            qb_pool = ep(tc.tile_pool(name="qb", bufs=2))
            qbs_pool = ep(tc.tile_pool(name="qbs", bufs=2))
            tmp_pool = ep(tc.tile_pool(name="tmp", bufs=2))
            vsb_pool = ep(tc.tile_pool(name="vsb", bufs=1))
            probs_pool = ep(tc.tile_pool(name="probs", bufs=4))
            woin_pool = ep(tc.tile_pool(name="woin", bufs=2))
            rec_pool = ep(tc.tile_pool(name="rec", bufs=2))
            rb_pool = ep(tc.tile_pool(name="rb", bufs=2))
            osb_pool = ep(tc.tile_pool(name="osb", bufs=3))
            # PSUM: sc 2x[128,1024] = 4 banks, ps 4x[128,512] = 4 banks.
            sc_pool = ep(tc.tile_pool(name="sc", bufs=2, space="PSUM"))
            ps_pool = ep(tc.tile_pool(name="ps", bufs=4, space="PSUM"))

            # ---- input DMAs, split fine and spread across issue queues ----
            wq_sb = consts.tile([128, 8, FPC], BF16, tag="wq")
            wk_sb = consts.tile([128, 8, FPC], BF16, tag="wk")
            wv_sb = consts.tile([128, 8, FPC], BF16, tag="wv")
            wq_r = wq_e.rearrange("(kt p) f -> p kt f", p=128)
            wk_r = wk_e.rearrange("(kt p) f -> p kt f", p=128)
            wv_r = wv_e.rearrange("(kt p) f -> p kt f", p=128)
            x_all = xin_pool.tile([128, 8, S], BF16, tag="x")

            # sync queue: wq interleaved with the first x quarter so the
            # q-projection starts streaming immediately; swap DMAs and
            # output DMAs ride this queue later (program order).
            for kt in range(8):
                nc.sync.dma_start(out=wq_sb[:, kt, :], in_=wq_r[:, kt, :])
                nc.sync.dma_start(out=x_all[:, kt, 0:512],
                                  in_=qkvT_e[kt * 128:(kt + 1) * 128, 0:512])
            for kt in range(8):
                nc.sync.dma_start(out=x_all[:, kt, 512:1024],
                                  in_=qkvT_e[kt * 128:(kt + 1) * 128,
                                             512:1024])

            # scalar queue: ONLY small/early DMAs — anything more clogs
            # the in-order Scalar queue and delays the exp stream (each
            # DMA issue occupies the queue ~0.6us and waits for slots).
            b_sbs = []
            for name, be in (("bq", bq_e), ("bk", bk_e)):
                t = consts.tile([128, 2], F32, tag=name, name=name)
                nc.scalar.dma_start(out=t[:],
                                    in_=be.rearrange("(t p) -> p t", p=128))
                b_sbs.append(t)
            bq_sb, bk_sb = b_sbs
            # only the h0 halves of the rope tables ride the scalar
            # queue (needed ~19us); h1 goes via gpsimd so the early rope
            # swap DMAs get scalar DMA-queue slots sooner.
            cos2_sb = consts.tile([128, S], BF16, tag="cos2")
            sinx_sb = consts.tile([128, S], BF16, tag="sinx")
            nc.scalar.dma_start(out=cos2_sb[:, 0:1024], in_=cos2_e[:, 0:1024])
            nc.scalar.dma_start(out=sinx_sb[:, 0:1024], in_=sinx_e[:, 0:1024])

            # gpsimd queue: bv + mask consts first, wv/wk (needed ~14us),
            # then x quarters 2,3 and the rest.
            bv_row = consts.tile([1, FPC], F32, tag="bvr")
            nc.gpsimd.dma_start(out=bv_row[:], in_=bv_e)
            nc.gpsimd.dma_start(out=cos2_sb[:, 1024:2048],
                                in_=cos2_e[:, 1024:2048])
            nc.gpsimd.dma_start(out=sinx_sb[:, 1024:2048],
                                in_=sinx_e[:, 1024:2048])
            mofs_sb = consts.tile([128, FPC], F32, tag="mofs")
            nc.gpsimd.dma_start(out=mofs_sb[:], in_=mofs_e)
            mlen_sb = consts.tile([128, 1], F32, tag="mlen")
            nc.gpsimd.dma_start(out=mlen_sb[:], in_=mlen_e)
            for kt in range(0, 8, 4):
                nc.gpsimd.dma_start(out=wv_sb[:, kt:kt + 4, :],
                                    in_=wv_r[:, kt:kt + 4, :])
            for kt in range(0, 8, 4):
                nc.gpsimd.dma_start(out=wk_sb[:, kt:kt + 4, :],
                                    in_=wk_r[:, kt:kt + 4, :])
            for kt in range(8):
                nc.gpsimd.dma_start(
                    out=x_all[:, kt, 1024:2048],
                    in_=qkvT_e[kt * 128:(kt + 1) * 128, 1024:2048])
            wo_sb = consts.tile([128, 2, D], BF16, tag="wo")
            wo_r = wo_e.rearrange("(pt p) f -> p pt f", p=128)
            for pt in range(2):
                nc.gpsimd.dma_start(out=wo_sb[:, pt, :], in_=wo_r[:, pt, :])
            bo_sb = consts.tile([128, 8], F32, tag="bo")
            nc.gpsimd.dma_start(out=bo_sb[:],
                                in_=bo_e.rearrange("(t p) -> p t", p=128))
            bv_bc = consts.tile([128, FPC], F32, tag="bvb")
            nc.gpsimd.partition_broadcast(bv_bc[:], bv_row[0:1, :])


            # v with ones row: [pos 128, 16 s-tiles, 4 heads, 64+1]
            v_sb = vsb_pool.tile([128, 16, 4, DH + 1], BF16, tag="v")
            nc.vector.memset(v_sb[:, :, :, DH:DH + 1], 1.0)

            # rope targets: [feat 128, S] per partition-tile, bf16
            qrot = [rope_pool.tile([128, S], BF16, tag="rope", name=f"qrot{i}")
                    for i in range(2)]
            krot = [rope_pool.tile([128, S], BF16, tag="rope", name=f"krot{i}")
                    for i in range(2)]

            def proj_st(qb, w_sb, b_sb, pt, st, on_act):
                ss = slice(st * 512, (st + 1) * 512)
                ps = ps_pool.tile([128, 512], F32, tag="ps", name="psp")
                for kt in range(8):
                    nc.tensor.matmul(
                        ps[:], w_sb[:, kt, pt * 128:(pt + 1) * 128],
                        x_all[:, kt, ss],
                        start=(kt == 0), stop=(kt == 7))
                if on_act:
                    nc.scalar.activation(out=qb[:, ss], in_=ps[:],
                                         func=IDENT,
                                         bias=b_sb[:, pt:pt + 1])
                else:
                    nc.vector.tensor_scalar_add(
                        out=qb[:, ss], in0=ps[:],
                        scalar1=b_sb[:, pt:pt + 1])

            def proj_block(w_sb, b_sb, pt, on_act):
                # q/k projection for one 128-feature block -> qb bf16
                qb = qb_pool.tile([128, S], BF16, tag="qb", name="qb")
                for st in range(NST):
                    proj_st(qb, w_sb, b_sb, pt, st, on_act)
                return qb

            def rope_span(qb, qbs, dst, lo, hi, early=False):
                # dst = qb*cos + swap32(qb)*sinx for one free-dim span.
                # Front ropes issue swaps on the gpsimd queue (its
                # broadcasts aren't needed until ~50us; on the scalar
                # queue the swap issues' data-waits would block the
                # first exps); later ones ride sync.
                eng = nc.gpsimd if early else nc.sync
                hs = slice(lo, hi)
                nc.vector.tensor_mul(out=dst[:, hs], in0=qb[:, hs],
                                     in1=cos2_sb[:, hs])
                for blk in (0, 1):
                    p0 = blk * 64
                    eng.dma_start(out=qbs[p0:p0 + 32, hs],
                                  in_=qb[p0 + 32:p0 + 64, hs])
                    eng.dma_start(out=qbs[p0 + 32:p0 + 64, hs],
                                  in_=qb[p0:p0 + 32, hs])
                tmp = tmp_pool.tile([128, hi - lo], BF16, tag="tmp",
                                    name="tmp")
                nc.vector.tensor_mul(out=tmp[:], in0=qbs[:, hs],
                                     in1=sinx_sb[:, hs])
                nc.vector.tensor_add(out=dst[:, hs], in0=dst[:, hs],
                                     in1=tmp[:])

            def rope_half(qb, qbs, dst, h, early=False):
                rope_span(qb, qbs, dst, h * 1024, (h + 1) * 1024, early)

            def v_block(sti):
                # v projected position-major: x chunk stationary, Wv moving
                pv_ps = ps_pool.tile([128, FPC], F32, tag="ps", name="pv_ps")
                for kt in range(8):
                    nc.tensor.matmul(
                        pv_ps[:], x_all[:, kt, sti * 128:(sti + 1) * 128],
                        wv_sb[:, kt, :],
                        start=(kt == 0), stop=(kt == 7))
                nc.vector.tensor_add(
                    out=v_sb[:, sti, :, 0:DH],
                    in0=pv_ps[:].rearrange("p (h d) -> p h d", h=4),
                    in1=bv_bc[:].rearrange("p (h d) -> p h d", h=4))

            woin = [woin_pool.tile([128, S], BF16, tag="woin",
                                   name=f"woin{i}") for i in range(2)]

            def scores(kt, qt, pair, qt0):
                ksl = slice(kt * 128, (kt + 1) * 128)
                off = max(0, kt * 128 - qt * 512)
                ps_s = sc_pool.tile([128, 1024], F32, tag="sc", name="ps_s")
                psv = ps_s[:].rearrange("p (h q) -> p h q", h=2)
                for h in (0, 1):
                    nc.tensor.matmul(
                        psv[:, h, off:512],
                        krot[pair][h * 64:(h + 1) * 64, ksl],
                        qrot[pair][h * 64:(h + 1) * 64, qt0 + off:qt0 + 512],
                        start=True, stop=True)
                pr = probs_pool.tile([128, 1024], BF16, tag="pr", name="pr")
                prv = pr[:].rearrange("p (h q) -> p h q", h=2)
                nc.scalar.activation(out=prv[:, :, off:512],
                                     in_=psv[:, :, off:512],
                                     func=EXP, scale=0.125)
                if kt * 128 >= qt * 512:
                    # zero probs above the diagonal of this 128-block:
                    # keep column qc (0..127 per head) iff qc >= partition;
                    # TENSOR_MASK: out = in0 if (in1 + imm2) < s0 else 0
                    from concourse.dve_ops import TENSOR_MASK
                    for h in (0, 1):
                        nc.vector._custom_dve(
                            TENSOR_MASK,
                            out=prv[:, h, off:off + 128],
                            in0=prv[:, h, off:off + 128],
                            in1=mofs_sb[:, 0:128],
                            s0=mlen_sb[:, 0:1],
                            imm2=0.0)
                return pr

            def pv(kt, pr, pv_a, pv_b, pair, nkt, qt):
                off = max(0, kt * 128 - qt * 512)
                prv = pr[:].rearrange("p (h q) -> p h q", h=2)
                nc.tensor.matmul(
                    pv_a[0:DH + 1, off:512], v_sb[:, kt, 2 * pair, :],
                    prv[:, 0, off:512],
                    start=(kt == 0), stop=(kt == nkt - 1))
                nc.tensor.matmul(
                    pv_b[0:DH + 1, off:512], v_sb[:, kt, 2 * pair + 1, :],
                    prv[:, 1, off:512],
                    start=(kt == 0), stop=(kt == nkt - 1))

            def attn_pair(qt, pair, fillers=()):
                # scores+pv pipeline for one (q-tile, head-pair). fillers
                # are zero-arg callables emitting independent PE work
                # (v blocks / next projection tiles), woven one per kt so
                # the PE stays busy while the exp stream catches up.
                fillers = list(fillers)
                qt0 = qt * 512
                qsl = slice(qt0, qt0 + 512)
                pv_a = ps_pool.tile([DH + 1, 512], F32, tag="ps",
                                    name="pv_a")
                pv_b = ps_pool.tile([DH + 1, 512], F32, tag="ps",
                                    name="pv_b")
                nkt = 4 * qt + 4
                pr_prev = scores(0, qt, pair, qt0)
                if fillers:
                    fillers.pop(0)()
                for kt in range(1, nkt):
                    pr_k = scores(kt, qt, pair, qt0)
                    if fillers:
                        fillers.pop(0)()
                    pv(kt - 1, pr_prev, pv_a, pv_b, pair, nkt, qt)
                    pr_prev = pr_k
                pv(nkt - 1, pr_prev, pv_a, pv_b, pair, nkt, qt)
                for f in fillers:
                    f()

                # denominator reciprocal + broadcast + normalize
                den = rec_pool.tile([1, 1024], F32, tag="den", name="den")
                nc.vector.tensor_copy(out=den[0:1, 0:512],
                                      in_=pv_a[DH:DH + 1, :])
                nc.vector.tensor_copy(out=den[0:1, 512:1024],
                                      in_=pv_b[DH:DH + 1, :])
                rec = rec_pool.tile([1, 1024], F32, tag="rec", name="rec")
                nc.vector.reciprocal_approx_fast(
                    out=rec[0:1, :], in_=den[0:1, :])
                rb = rb_pool.tile([128, 1024], F32, tag="rb", name="rb")
                nc.gpsimd.partition_broadcast(rb[:], rec[0:1, :])
                nc.vector.tensor_mul(out=woin[pair][0:64, qsl],
                                     in0=pv_a[0:DH, :],
                                     in1=rb[0:64, 0:512])
                nc.vector.tensor_mul(out=woin[pair][64:128, qsl],
                                     in0=pv_b[0:DH, :],
                                     in1=rb[64:128, 512:1024])

            def wo_block(qt, last=False):
                qsl = slice(qt * 512, (qt + 1) * 512)
                for dm in range(8):
                    ps_o = ps_pool.tile([128, 512], F32, tag="ps",
                                        name="ps_o")
                    for pt in range(2):
                        nc.tensor.matmul(
                            ps_o[:], wo_sb[:, pt, dm * 128:(dm + 1) * 128],
                            woin[pt][:, qsl], start=(pt == 0), stop=(pt == 1))
                    ot = osb_pool.tile([128, QT], BF16, tag="ot", name="ot")
                    if last and dm % 2 == 1:
                        nc.scalar.activation(out=ot[:], in_=ps_o[:],
                                             func=IDENT,
                                             bias=bo_sb[:, dm:dm + 1])
                    else:
                        nc.vector.tensor_scalar_add(
                            out=ot[:], in0=ps_o[:],
                            scalar1=bo_sb[:, dm:dm + 1])
                    if last:
                        q0_ = qt * 512
                        nc.sync.dma_start(
                            out=out_e[dm * 128:(dm + 1) * 128,
                                      q0_:q0_ + 256], in_=ot[:, 0:256])
                        nc.sync.dma_start(
                            out=out_e[dm * 128:(dm + 1) * 128,
                                      q0_ + 256:q0_ + 512], in_=ot[:, 256:512])
                    else:
                        nc.sync.dma_start(
                            out=out_e[dm * 128:(dm + 1) * 128, qsl],
                            in_=ot[:])

            # ---- interleaved schedule ----
            # Fine-grained front: the first position-halves of the q0/k0
            # projections and their RoPE go first so qt0-pair0 scores (and
            # the exp stream) start ~20us in. Everything else — remaining
            # projection tiles, RoPE halves, v blocks — is woven into the
            # pair-0 attention as PE filler. Then pair 1 of every q-tile
            # (order 3,0,1,2) with the Wo blocks one step behind.
            qbq0 = qb_pool.tile([128, S], BF16, tag="qb", name="qb_q0")
            qbk0 = qb_pool.tile([128, S], BF16, tag="qb", name="qb_k0")
            qsq0 = qbs_pool.tile([128, S], BF16, tag="qbs", name="qs_q0")
            qsk0 = qbs_pool.tile([128, S], BF16, tag="qbs", name="qs_k0")
            qbq1 = qb_pool.tile([128, S], BF16, tag="qbx", name="qb_q1")
            qbk1 = qb_pool.tile([128, S], BF16, tag="qbx", name="qb_k1")
            qsq1 = qbs_pool.tile([128, S], BF16, tag="qbsx", name="qs_q1")
            qsk1 = qbs_pool.tile([128, S], BF16, tag="qbsx", name="qs_k1")

            # Dummy matmuls on the first wq chunk fill the x-DMA wait
            # gaps so the HAM clock-gate stays open through the front
            # (otherwise the whole front runs at 1.2 GHz).
            warm_ps = ps_pool.tile([128, 512], F32, tag="ps", name="warm")
            warm_rhs = wq_sb[:, 0:2, :].rearrange("p a b -> p (a b)")

            def warm(n):
                for _ in range(n):
                    nc.tensor.matmul(warm_ps[:], wq_sb[:, 0, 0:128],
                                     warm_rhs[:, 0:512],
                                     start=True, stop=True)

            warm(4)
            proj_st(qbq0, wq_sb, bq_sb, 0, 0, False)
            rope_span(qbq0, qsq0, qrot[0], 0, 512, early=True)
            warm(3)
            v_block(0)
            warm(2)
            v_block(1)
            proj_st(qbq0, wq_sb, bq_sb, 0, 1, False)
            rope_span(qbq0, qsq0, qrot[0], 512, 1024, early=True)
            warm(2)
            v_block(2)
            v_block(3)
            proj_st(qbk0, wk_sb, bk_sb, 0, 0, False)
            rope_span(qbk0, qsk0, krot[0], 0, 512, early=True)
            warm(2)
            v_block(4)
            v_block(5)
            proj_st(qbk0, wk_sb, bk_sb, 0, 1, False)
            rope_span(qbk0, qsk0, krot[0], 512, 1024, early=True)
            v_block(6)
            v_block(7)

            fill = [
                # qt0 pair0 (4 kts)
                lambda: proj_st(qbq0, wq_sb, bq_sb, 0, 2, True),
                lambda: proj_st(qbk0, wk_sb, bk_sb, 0, 2, True),
                lambda: v_block(8),
                lambda: v_block(9),
                # qt1 pair0 (8 kts)
                lambda: proj_st(qbq0, wq_sb, bq_sb, 0, 3, True),
                lambda: proj_st(qbk0, wk_sb, bk_sb, 0, 3, True),
                lambda: rope_half(qbq0, qsq0, qrot[0], 1),
                lambda: rope_half(qbk0, qsk0, krot[0], 1),
                lambda: v_block(10),
                lambda: v_block(11),
                # qt2 pair0 (12 kts): v12..15 first (qt3's pv needs them)
                lambda: v_block(12),
                lambda: v_block(13),
                lambda: v_block(14),
                lambda: v_block(15),
                lambda: proj_st(qbq1, wq_sb, bq_sb, 1, 0, False),
                lambda: proj_st(qbq1, wq_sb, bq_sb, 1, 1, False),
                lambda: rope_half(qbq1, qsq1, qrot[1], 0),
                lambda: proj_st(qbq1, wq_sb, bq_sb, 1, 2, False),
                lambda: proj_st(qbq1, wq_sb, bq_sb, 1, 3, False),
                lambda: rope_half(qbq1, qsq1, qrot[1], 1),
                # qt3 pair0 (16 kts): k1 h0 only — k1 h1 moves into the
                # pair-1 phase where the PE otherwise idles
                lambda: proj_st(qbk1, wk_sb, bk_sb, 1, 0, False),
                lambda: proj_st(qbk1, wk_sb, bk_sb, 1, 1, False),
                lambda: rope_half(qbk1, qsk1, krot[1], 0),
            ]
            attn_pair(0, 0, fillers=fill[0:4])
            attn_pair(1, 0, fillers=fill[4:10])
            attn_pair(2, 0, fillers=fill[10:20])
            attn_pair(3, 0, fillers=fill[20:])

            fill1 = [
                lambda: proj_st(qbk1, wk_sb, bk_sb, 1, 2, False),
                lambda: proj_st(qbk1, wk_sb, bk_sb, 1, 3, False),
                lambda: rope_half(qbk1, qsk1, krot[1], 1),
            ]

            attn_pair(0, 1, fillers=fill1)
            attn_pair(3, 1)
            wo_block(0)
            attn_pair(2, 1)
            wo_block(3)
            attn_pair(1, 1)
            wo_block(2)
            wo_block(1, last=True)

    nc.compile()
    return nc


def kernel(qkv, cos, sin, Wq, bq, Wk, bk, Wv, bv, Wo, bo):
    from concourse.bass_utils import run_bass_kernel_spmd

    qkv = np.asarray(qkv, dtype=np.float32)
    cos = np.asarray(cos, dtype=np.float32)
    sin = np.asarray(sin, dtype=np.float32)
    Wq, bq = np.asarray(Wq, np.float32), np.asarray(bq, np.float32)
    Wk, bk = np.asarray(Wk, np.float32), np.asarray(bk, np.float32)
    Wv, bv = np.asarray(Wv, np.float32), np.asarray(bv, np.float32)
    Wo, bo = np.asarray(Wo, np.float32), np.asarray(bo, np.float32)

    if "nc" not in _cache:
        _cache["nc"] = _build()
    nc = _cache["nc"]

    bf = ml_dtypes.bfloat16
    cos2 = np.ascontiguousarray(np.tile(cos.T, (2, 1)).astype(bf))  # [128, S]
    sinx = np.tile(sin.T, (2, 1))
    sinx[0:32] *= -1.0
    sinx[64:96] *= -1.0
    sinx = np.ascontiguousarray(sinx.astype(bf))

    # tensor_mask consts: keep col qc (per 128-head-block) iff qc >= p
    #   keep iff mofs[p, qc] + 0 < mlen[p]  with mofs = -qc, mlen = 1-p
    qc = np.arange(128, dtype=np.float32)
    mofs = np.ascontiguousarray(np.tile(-qc, (128, 2)))  # [128, 256]
    mlen = np.ascontiguousarray((1.0 - qc)[:, None])  # [128, 1]

    bo4 = np.ascontiguousarray(bo * 0.25)
    in_maps = []
    for c in range(N_CORES):
        b, g = c // 4, c % 4
        hsl = slice(g * FPC, (g + 1) * FPC)
        in_maps.append({
            "qkvT": np.ascontiguousarray(qkv[b].T.astype(bf)),
            "wq": np.ascontiguousarray(Wq[hsl, :].T.astype(bf)),
            "wk": np.ascontiguousarray(Wk[hsl, :].T.astype(bf)),
            "wv": np.ascontiguousarray(Wv[hsl, :].T.astype(bf)),
            "wo": np.ascontiguousarray(Wo[:, hsl].T.astype(bf)),
            "bq": np.ascontiguousarray(bq[hsl]),
            "bk": np.ascontiguousarray(bk[hsl]),
            "bv": np.ascontiguousarray(bv[hsl][None, :]),
            "bo": bo4,
            "cos2": cos2,
            "sinx": sinx,
            "mofs": mofs,
            "mlen": mlen,
        })

    trace = bool(os.environ.get("KERNEL_TRACE"))
    res = run_bass_kernel_spmd(nc, in_maps, list(range(N_CORES)), trace=trace)
    last_run_info["exec_time_ns"] = res.exec_time_ns
    last_run_info["results"] = res

    out = np.empty((B, S, D), dtype=np.float32)
    for b in range(B):
        oT = (res.results[4 * b]["out"].astype(np.float32)
              + res.results[4 * b + 1]["out"].astype(np.float32)
              + res.results[4 * b + 2]["out"].astype(np.float32)
              + res.results[4 * b + 3]["out"].astype(np.float32))
        out[b] = oT.T
    return out
